# revision 1
# baseline (speedup 1.0000x reference)
"""Trainium2 Bass kernel for the FFT-block (attention + conv FFN) problem.

Sharding: data-parallel over batch. B=16 items across 8 cores -> 2 items/core.
Each core runs the full block for its items; no collectives.

Per item:
  - attention via scores^T = K Q^T (softmax sums land on the partition axis and
    are folded into the ctx matmul through a ones-column appended to V); the
    per-head 1/Z normalization is broadcast across partitions with a K=1 PE
    matmul.  Attention matmuls run in fp32r (tf32-like, fp32 accumulate);
    softmax weights and V are bf16.
  - convs are 9 shifted matmuls over transposed activations hT [D, S_pad] in
    bf16 (weights+activations), fp32 PSUM accumulation and fp32 o2 accumulator.
  - emission order software-pipelines item1's attention into item0's conv
    stream so the PE never drains (HAM stays at K=8/8).
"""
import sys, types
import numpy as np

B, S, D = 16, 1024, 512
H, DK = 8, 64
CD, KS = 2048, 9
EPS = 1e-5
NCORES = 8
NIT = B // NCORES
NDC = D // 128             # 4 d-chunks
NSC = S // 128             # 8 s-chunks
NCOL = S // 512            # 2 s-cols
NCD = CD // 128            # 16 cd-chunks


def _install_ntff_hook():
    try:
        from antenv.axon_hooks import get_axon_ntff_profile_hook  # noqa
        return
    except ImportError:
        pass
    try:
        from trn_agent_boot.trn_boot import _ntff_profile_via_ctypes
        mod = types.ModuleType('antenv.axon_hooks')
        hook = _ntff_profile_via_ctypes('/opt/axon/libaxon_pjrt.so')
        mod.get_axon_ntff_profile_hook = lambda: hook
        sys.modules['antenv.axon_hooks'] = mod
    except Exception:
        pass


_BUILT = None


def _build():
    global _BUILT
    if _BUILT is not None:
        return _BUILT
    _install_ntff_hook()
    import concourse.bacc as bacc
    import concourse.mybir as mybir
    from concourse import tile
    from concourse.masks import make_identity
    from contextlib import ExitStack

    F32 = mybir.dt.float32
    F32R = mybir.dt.float32r
    BF16 = mybir.dt.bfloat16
    AF = mybir.ActivationFunctionType
    ALU = mybir.AluOpType
    AX = mybir.AxisListType

    nc = bacc.Bacc("TRN2", target_bir_lowering=False, debug=False,
                   num_devices=NCORES)

    # ---- DRAM I/O (per core) ----
    d_xT = nc.dram_tensor("xT", [NIT, NDC, 128, S], F32, kind="ExternalInput")
    d_xp = nc.dram_tensor("xp", [NIT, NSC, 128, D], F32, kind="ExternalInput")
    d_wqk = nc.dram_tensor("wqk", [2, 4, 128, 512], F32, kind="ExternalInput")
    d_bqk = nc.dram_tensor("bqk", [128, 8], F32, kind="ExternalInput")
    d_wv = nc.dram_tensor("wv", [NDC, 128, 520], F32, kind="ExternalInput")
    d_bvrow = nc.dram_tensor("bvrow", [128, 520], F32, kind="ExternalInput")
    d_wo = nc.dram_tensor("wo", [4, 128, 512], F32, kind="ExternalInput")
    d_w1 = nc.dram_tensor("w1", [NCD, NDC, 128, KS * 128], BF16,
                          kind="ExternalInput")
    d_w2 = nc.dram_tensor("w2", [NCD, 128, KS * 512], BF16,
                          kind="ExternalInput")
    d_bc1s = nc.dram_tensor("bc1s", [128, NCD], F32, kind="ExternalInput")
    d_gb = nc.dram_tensor("gb", [5, 128, 512], F32, kind="ExternalInput")
    d_cones = nc.dram_tensor("cones", [128, 128], F32, kind="ExternalInput")
    d_czero = nc.dram_tensor("czero", [128, 8], BF16, kind="ExternalInput")
    d_y = nc.dram_tensor("y", [NIT, NSC, 128, D], F32, kind="ExternalOutput")

    G1, B1, G2, B2, BC2 = range(5)

    with tile.TileContext(nc) as tc:
        est = ExitStack()
        with est:
            cp = est.enter_context(tc.tile_pool(name="const", bufs=1))
            pl = est.enter_context(tc.tile_pool(name="work", bufs=1))
            ps = est.enter_context(tc.tile_pool(name="psum", bufs=1, space="PSUM"))
            dp = est.enter_context(tc.tile_pool(name="dramp", bufs=1, space="DRAM"))

            h_dram = [[dp.tile([128, D], F32, tag=f"hd{it}_{sc}",
                               name=f"hd{it}_{sc}")
                       for sc in range(NSC)] for it in range(NIT)]

            # ---- constants ----
            t_bqk = cp.tile([128, 8], F32, tag="bqk")
            nc.sync.dma_start(t_bqk[:], d_bqk[:])
            t_gb = []
            for i in range(5):
                t = cp.tile([128, 512], F32, tag=f"gb{i}", name=f"gb{i}")
                nc.sync.dma_start(t[:], d_gb[i])
                t_gb.append(t)
            t_bc1s = cp.tile([128, NCD], F32, tag="bc1s")
            nc.sync.dma_start(t_bc1s[:], d_bc1s[:])
            t_ident = cp.tile([128, 128], F32, tag="ident")
            make_identity(nc, t_ident[:])
            t_cones = cp.tile([128, 128], F32R, tag="cones")
            nc.sync.dma_start(t_cones[:], d_cones[:].bitcast(F32R))
            t_czero = cp.tile([128, 8], BF16, tag="czero")
            nc.sync.dma_start(t_czero[:], d_czero[:])
            t_eps = cp.tile([128, 1], F32, tag="eps")
            nc.vector.memset(t_eps[:], EPS)
            t_wv = []
            for dc in range(NDC):
                t = cp.tile([128, 520], F32R, tag=f"wv{dc}", name=f"wv{dc}")
                nc.sync.dma_start(t[:], d_wv[dc].bitcast(F32R))
                t_wv.append(t)
            t_bvrow = cp.tile([128, 520], F32R, tag="bvrow")
            nc.sync.dma_start(t_bvrow[:], d_bvrow[:].bitcast(F32R))
            t_wo = []
            for c in range(4):
                t = cp.tile([128, 512], F32R, tag=f"wo{c}", name=f"wo{c}")
                nc.sync.dma_start(t[:], d_wo[c].bitcast(F32R))
                t_wo.append(t)

            # persistent hT tiles (bf16, padded s)
            hT = [[pl.tile([128, S + 8], BF16, tag=f"ht{it}_{dc}",
                           name=f"ht{it}_{dc}")
                   for dc in range(NDC)] for it in range(NIT)]

            state = [dict() for _ in range(NIT)]

            # ================= emit helpers =================
            def emit_x(it):
                st = state[it]
                xt = []
                for dc in range(NDC):
                    t = pl.tile([128, S], F32R, tag=f"xt{dc}", name=f"xt{dc}")
                    nc.sync.dma_start(t[:], d_xT[it, dc].bitcast(F32R))
                    xt.append(t)
                st["xt"] = xt
                st["qkt"] = {}

            def emit_v(it):
                """V projection for one item (dense PE block)."""
                st = state[it]
                xt = st["xt"]
                vst = []
                for tc_i in range(NSC):
                    vt = pl.tile([128, 520], BF16, tag=f"vst{tc_i}",
                                 name=f"vst{tc_i}")
                    for half in range(2):
                        colo = half * 260
                        pv = ps.tile([128, 260], F32, tag="pp", bufs=3)
                        for dc in range(NDC):
                            nc.tensor.matmul(
                                pv[:], xt[dc][:, tc_i * 128:(tc_i + 1) * 128],
                                t_wv[dc][:, colo:colo + 260],
                                start=(dc == 0), stop=False)
                        nc.tensor.matmul(
                            pv[:], t_cones[0:1, 0:128],
                            t_bvrow[0:1, colo:colo + 260],
                            start=False, stop=True)
                        nc.vector.tensor_copy(vt[:, colo:colo + 260], pv[:])
                    vst.append(vt)
                st["vst"] = vst

            def emit_qk(it, pair):
                st = state[it]
                xt = st["xt"]
                for proj in range(2):
                    wt = pl.tile([128, 512], F32R, tag=f"wqk{proj}",
                                 bufs=2, name="wt")
                    nc.sync.dma_start(wt[:], d_wqk[proj, pair].bitcast(F32R))
                    qt = pl.tile([128, S], BF16, tag=f"qk{proj}{pair}",
                                 name="qt")
                    for scol in range(NCOL):
                        pq = ps.tile([128, 512], F32, tag="pp", bufs=3)
                        for dc in range(NDC):
                            nc.tensor.matmul(
                                pq[:], wt[:, dc * 128:(dc + 1) * 128],
                                xt[dc][:, scol * 512:(scol + 1) * 512],
                                start=(dc == 0), stop=(dc == NDC - 1))
                        nc.vector.tensor_scalar_add(
                            qt[:, scol * 512:(scol + 1) * 512], pq[:],
                            t_bqk[:, proj * 4 + pair:proj * 4 + pair + 1])
                    st["qkt"][(proj, pair)] = qt

            def emit_heads_pair(it, pair):
                st = state[it]
                if pair == 0:
                    st["ctxT"] = [pl.tile([128, S], F32R, tag=f"ct{c}",
                                          name=f"ct{c}") for c in range(4)]
                qT = st["qkt"][(0, pair)]
                kT = st["qkt"][(1, pair)]
                vst = st["vst"]
                ctxT = st["ctxT"]
                for sub in range(2):
                    h = 2 * pair + sub
                    hr = slice(sub * 64, sub * 64 + 64)
                    for scol in range(NCOL):
                        so = scol * 512
                        pex = []
                        for ti in range(NSC):
                            pp = ps.tile([128, 512], F32, tag="pp", bufs=3)
                            nc.tensor.matmul(
                                pp[:], kT[hr, ti * 128:(ti + 1) * 128],
                                qT[hr, so:so + 512], start=True, stop=True)
                            pe = pl.tile([128, 512], BF16, tag=f"pex{ti}",
                                         bufs=1, name="pe")
                            nc.scalar.activation(pe[:], pp[:], AF.Exp,
                                                 scale=0.125)
                            pex.append(pe)
                        pc = ps.tile([65, 512], F32, tag="pc", bufs=1)
                        for ti in range(NSC):
                            nc.tensor.matmul(
                                pc[:], vst[ti][:, h * 65:h * 65 + 65],
                                pex[ti][:], start=(ti == 0),
                                stop=(ti == NSC - 1))
                        zr = pl.tile([64, 512], F32R, tag="bcs", bufs=2,
                                     name="zr")
                        nc.vector.tensor_copy(zr[0:1, :], pc[64:65, :])
                        pb = ps.tile([64, 512], F32, tag="pp", bufs=3)
                        nc.tensor.matmul(pb[:], t_cones[0:1, 0:64], zr[0:1, :],
                                         start=True, stop=True)
                        bcs = pl.tile([64, 512], F32, tag="bcs", bufs=2,
                                      name="bcs")
                        nc.vector.reciprocal_approx_fast(out=bcs[:], in_=pb[:])
                        nc.vector.tensor_tensor(
                            ctxT[pair][hr, so:so + 512], pc[0:64, :],
                            bcs[:], ALU.mult)


            def emit_tail(it):
                """Wo + residual + LN1 + transpose into hT (+ h spill)."""
                st = state[it]
                ctxT = st["ctxT"]
                st_sum = pl.tile([128, NSC], F32, tag="st_sum", bufs=2)
                st_sq = pl.tile([128, NSC], F32, tag="st_sq", bufs=2)
                rr = []
                for sc in range(NSC):
                    xpt = pl.tile([128, 512], F32, tag="xpt", bufs=2)
                    nc.sync.dma_start(xpt[:], d_xp[it, sc])
                    pw = ps.tile([128, 512], F32, tag="pc", bufs=1)
                    for c in range(4):
                        nc.tensor.matmul(
                            pw[:], ctxT[c][:, sc * 128:(sc + 1) * 128],
                            t_wo[c][:], start=(c == 0), stop=(c == 3))
                    r = pl.tile([128, 512], F32, tag=f"res{sc}", name="r")
                    nc.vector.tensor_tensor(r[:], pw[:], xpt[:], ALU.add)
                    nc.vector.reduce_sum(st_sum[:, sc:sc + 1], r[:], axis=AX.X)
                    sq = pl.tile([128, 512], BF16, tag="sqs", bufs=2, name="sq")
                    nc.scalar.activation(sq[:], r[:], AF.Square,
                                         accum_out=st_sq[:, sc:sc + 1])
                    rr.append(r)
                mean8 = pl.tile([128, NSC], F32, tag="mean8", bufs=2)
                inv8 = pl.tile([128, NSC], F32, tag="inv8", bufs=2)
                msq = pl.tile([128, NSC], F32, tag="msq", bufs=2)
                nc.vector.tensor_scalar_mul(mean8[:], st_sum[:], 1.0 / D)
                nc.vector.tensor_scalar_mul(inv8[:], st_sq[:], 1.0 / D)
                nc.vector.tensor_tensor(msq[:], mean8[:], mean8[:], ALU.mult)
                nc.vector.tensor_tensor(inv8[:], inv8[:], msq[:], ALU.subtract)
                nc.scalar.activation(inv8[:], inv8[:], AF.Sqrt, bias=t_eps[:])
                nc.vector.reciprocal(inv8[:], inv8[:])
                for sc in range(NSC):
                    ht_ = pl.tile([128, 512], F32, tag="hst", bufs=2, name="h_")
                    nc.vector.tensor_scalar(
                        ht_[:], rr[sc][:], mean8[:, sc:sc + 1],
                        inv8[:, sc:sc + 1], ALU.subtract, ALU.mult)
                    nc.vector.tensor_tensor(ht_[:], ht_[:], t_gb[G1][:], ALU.mult)
                    nc.vector.tensor_tensor(ht_[:], ht_[:], t_gb[B1][:], ALU.add)
                    nc.sync.dma_start(h_dram[it][sc][:], ht_[:])
                    for dc in range(NDC):
                        pt = ps.tile([128, 128], F32, tag="pp", bufs=3)
                        nc.tensor.transpose(pt[:], ht_[:, dc * 128:(dc + 1) * 128],
                                            t_ident[:])
                        nc.scalar.copy(
                            hT[it][dc][:, 4 + sc * 128: 4 + (sc + 1) * 128],
                            pt[:])
                for dc in range(NDC):
                    nc.sync.dma_start(hT[it][dc][:, 0:4], d_czero[:, 0:4])
                    nc.sync.dma_start(hT[it][dc][:, S + 4:S + 8],
                                      d_czero[:, 4:8])

            o2 = [[None] * NSC for _ in range(NIT)]

            def emit_conv_chunk(it, cdc):
                w2t = pl.tile([128, KS * 512], BF16, tag="w2t", bufs=2,
                              name="w2t")
                nc.sync.dma_start(w2t[:], d_w2[cdc])
                w1t = []
                for dc in range(NDC):
                    t = pl.tile([128, KS * 128], BF16, tag=f"w1t{dc}", bufs=2,
                                name="w1t")
                    nc.sync.dma_start(t[:], d_w1[cdc, dc])
                    w1t.append(t)
                c1t = pl.tile([128, S + 8], BF16, tag="c1t", bufs=2, name="c1t")
                nc.sync.dma_start(c1t[:, 0:4], d_czero[:, 0:4])
                nc.sync.dma_start(c1t[:, S + 4:S + 8], d_czero[:, 4:8])
                for scol in range(NCOL):
                    pc1 = ps.tile([128, 512], F32, tag="c1p", bufs=2)
                    idx = 0
                    for k in range(KS):
                        for dc in range(NDC):
                            nc.tensor.matmul(
                                pc1[:], w1t[dc][:, k * 128:(k + 1) * 128],
                                hT[it][dc][:, scol * 512 + k:
                                           scol * 512 + k + 512],
                                start=(idx == 0), stop=(idx == 35))
                            idx += 1
                    nc.scalar.activation(
                        c1t[:, 4 + scol * 512: 4 + (scol + 1) * 512],
                        pc1[:], AF.Relu, bias=t_bc1s[:, cdc:cdc + 1])
                for sc in range(NSC):
                    pc2 = ps.tile([128, 512], F32, tag="c2p", bufs=2)
                    for k in range(KS):
                        nc.tensor.matmul(
                            pc2[:], c1t[:, sc * 128 + k: sc * 128 + k + 128],
                            w2t[:, k * 512:(k + 1) * 512],
                            start=(k == 0), stop=(k == KS - 1))
                    if cdc == 0:
                        t = pl.tile([128, 512], F32, tag=f"o2_{sc}",
                                    name=f"o2_{sc}")
                        o2[it][sc] = t
                        nc.vector.tensor_copy(t[:], pc2[:])
                    else:
                        nc.vector.tensor_tensor(o2[it][sc][:], pc2[:],
                                                o2[it][sc][:], ALU.add)

            def emit_ln2(it):
                st_sum = pl.tile([128, NSC], F32, tag="st_sum", bufs=2)
                st_sq = pl.tile([128, NSC], F32, tag="st_sq", bufs=2)
                rr = []
                for sc in range(NSC):
                    t1 = pl.tile([128, 512], F32, tag="hst", bufs=2)
                    nc.vector.tensor_tensor(t1[:], o2[it][sc][:], t_gb[BC2][:],
                                            ALU.add)
                    nc.scalar.activation(t1[:], t1[:], AF.Relu)
                    hrl = pl.tile([128, 512], F32, tag="xpt", bufs=2)
                    nc.sync.dma_start(hrl[:], h_dram[it][sc][:])
                    r = pl.tile([128, 512], F32, tag=f"res{sc}", name="r2")
                    nc.vector.tensor_tensor(r[:], t1[:], hrl[:], ALU.add)
                    nc.vector.reduce_sum(st_sum[:, sc:sc + 1], r[:], axis=AX.X)
                    sq = pl.tile([128, 512], BF16, tag="sqs", bufs=2, name="sq2")
                    nc.scalar.activation(sq[:], r[:], AF.Square,
                                         accum_out=st_sq[:, sc:sc + 1])
                    rr.append(r)
                mean8 = pl.tile([128, NSC], F32, tag="mean8", bufs=2)
                inv8 = pl.tile([128, NSC], F32, tag="inv8", bufs=2)
                msq = pl.tile([128, NSC], F32, tag="msq", bufs=2)
                nc.vector.tensor_scalar_mul(mean8[:], st_sum[:], 1.0 / D)
                nc.vector.tensor_scalar_mul(inv8[:], st_sq[:], 1.0 / D)
                nc.vector.tensor_tensor(msq[:], mean8[:], mean8[:], ALU.mult)
                nc.vector.tensor_tensor(inv8[:], inv8[:], msq[:], ALU.subtract)
                nc.scalar.activation(inv8[:], inv8[:], AF.Sqrt, bias=t_eps[:])
                nc.vector.reciprocal(inv8[:], inv8[:])
                for sc in range(NSC):
                    yt = pl.tile([128, 512], F32, tag="hst", bufs=2)
                    nc.vector.tensor_scalar(
                        yt[:], rr[sc][:], mean8[:, sc:sc + 1],
                        inv8[:, sc:sc + 1], ALU.subtract, ALU.mult)
                    nc.vector.tensor_tensor(yt[:], yt[:], t_gb[G2][:], ALU.mult)
                    nc.vector.tensor_tensor(yt[:], yt[:], t_gb[B2][:], ALU.add)
                    nc.sync.dma_start(d_y[it, sc], yt[:])

            # ================= emission order =================
            emit_x(0)
            emit_v(0)
            for pair in range(4):
                emit_qk(0, pair)
            emit_x(1)
            for pair in range(4):
                emit_heads_pair(0, pair)
                emit_qk(1, pair)
            emit_v(1)
            emit_tail(0)
            for cdc in range(NCD):
                emit_conv_chunk(0, cdc)
                if cdc < 4:
                    emit_heads_pair(1, cdc)
                elif cdc == 7:
                    emit_tail(1)
            emit_ln2(0)
            for cdc in range(NCD):
                emit_conv_chunk(1, cdc)
            emit_ln2(1)

    nc.compile()
    _BUILT = nc
    return nc


def _prep_host(inputs):
    import ml_dtypes
    bf16 = ml_dtypes.bfloat16
    x = np.asarray(inputs["x"], np.float32)
    Wq = np.asarray(inputs["Wq"], np.float32)
    bq = np.asarray(inputs["bq"], np.float32)
    Wk = np.asarray(inputs["Wk"], np.float32)
    bk = np.asarray(inputs["bk"], np.float32)
    Wv = np.asarray(inputs["Wv"], np.float32)
    bv = np.asarray(inputs["bv"], np.float32)
    Wo = np.asarray(inputs["Wo"], np.float32)
    bo = np.asarray(inputs["bo"], np.float32)
    g1 = np.asarray(inputs["g1"], np.float32)
    b1 = np.asarray(inputs["b1"], np.float32)
    g2 = np.asarray(inputs["g2"], np.float32)
    b2 = np.asarray(inputs["b2"], np.float32)
    Wc1 = np.asarray(inputs["Wc1"], np.float32)
    bc1 = np.asarray(inputs["bc1"], np.float32)
    Wc2 = np.asarray(inputs["Wc2"], np.float32)
    bc2 = np.asarray(inputs["bc2"], np.float32)

    xT = np.ascontiguousarray(x.transpose(0, 2, 1).reshape(B, NDC, 128, S))
    xp = np.ascontiguousarray((x + bo[None, None, :]).reshape(B, NSC, 128, D))

    wqk = np.zeros((2, 4, 128, 512), np.float32)
    for proj, W in ((0, Wq), (1, Wk)):
        for pair in range(4):
            blk = np.concatenate([W[2 * pair], W[2 * pair + 1]], axis=1)
            wqk[proj, pair] = blk.reshape(NDC, 128, 128).transpose(1, 0, 2) \
                                 .reshape(128, 512)
    bqk = np.zeros((128, 8), np.float32)
    for proj, b in ((0, bq), (1, bk)):
        for pair in range(4):
            bqk[:, proj * 4 + pair] = np.concatenate(
                [b[2 * pair], b[2 * pair + 1]])

    wv = np.zeros((NDC, 128, 520), np.float32)
    bvrow = np.zeros((128, 520), np.float32)
    for h in range(H):
        wv[:, :, h * 65:h * 65 + 64] = Wv[h].reshape(NDC, 128, 64)
        bvrow[0, h * 65:h * 65 + 64] = bv[h]
        bvrow[0, h * 65 + 64] = 1.0

    wo = np.ascontiguousarray(Wo.reshape(4, 128, 512))

    w1 = np.ascontiguousarray(
        Wc1.reshape(NCD, 128, NDC, 128, KS).transpose(0, 2, 3, 4, 1)
           .reshape(NCD, NDC, 128, KS * 128)).astype(bf16)
    w2 = np.ascontiguousarray(
        Wc2.reshape(D, NCD, 128, KS).transpose(1, 2, 3, 0)
           .reshape(NCD, 128, KS * 512)).astype(bf16)
    bc1s = np.ascontiguousarray(bc1.reshape(NCD, 128).T)

    gb = np.stack([np.tile(v[None, :], (128, 1))
                   for v in (g1, b1, g2, b2, bc2)]).astype(np.float32)
    cones = np.ones((128, 128), np.float32)
    czero = np.zeros((128, 8), bf16)

    shared = dict(wqk=wqk, bqk=bqk, wv=wv, bvrow=bvrow, wo=wo,
                  w1=w1, w2=w2, bc1s=bc1s, gb=gb, cones=cones, czero=czero)
    in_maps = []
    for c in range(NCORES):
        m = dict(shared)
        m["xT"] = np.ascontiguousarray(xT[c * NIT:(c + 1) * NIT])
        m["xp"] = np.ascontiguousarray(xp[c * NIT:(c + 1) * NIT])
        in_maps.append(m)
    return in_maps


def run(inputs, trace=False, **trace_kwargs):
    nc = _build()
    from concourse.bass_utils import run_bass_kernel_spmd
    in_maps = _prep_host(inputs)
    res = run_bass_kernel_spmd(nc, in_maps, core_ids=list(range(NCORES)),
                               trace=trace, **trace_kwargs)
    y = np.concatenate([res.results[c]["y"].reshape(NIT, S, D)
                        for c in range(NCORES)], axis=0)
    return y, res


def kernel(**inputs):
    y, _ = run(inputs, trace=False)
    return y



# revision 20
# speedup vs baseline: 1.0332x; 1.0332x over previous
"""Trainium2 Bass kernel for the FFT-block (attention + conv FFN) problem.

Sharding: data-parallel over batch. B=16 items across 8 cores -> 2 items/core.
Each core runs the full block for its items; no collectives.

Per item:
  - attention via scores^T = K Q^T (softmax sums land on the partition axis and
    are folded into the ctx matmul through a ones-column appended to V); the
    per-head 1/Z normalization is broadcast across partitions with a K=1 PE
    matmul.  Attention matmuls run in fp32r (tf32-like, fp32 accumulate);
    softmax weights and V are bf16.
  - convs are 9 shifted matmuls over transposed activations hT [D, S_pad] in
    bf16 (weights+activations), fp32 PSUM accumulation and fp32 o2 accumulator.
  - emission order software-pipelines item1's attention into item0's conv
    stream so the PE never drains (HAM stays at K=8/8).
"""
import sys, types
import numpy as np

B, S, D = 16, 1024, 512
H, DK = 8, 64
CD, KS = 2048, 9
EPS = 1e-5
NCORES = 8
NIT = B // NCORES
NDC = D // 128             # 4 d-chunks
NSC = S // 128             # 8 s-chunks
NCOL = S // 512            # 2 s-cols
NCD = CD // 128            # 16 cd-chunks


def _install_ntff_hook():
    try:
        from antenv.axon_hooks import get_axon_ntff_profile_hook  # noqa
        return
    except ImportError:
        pass
    try:
        from trn_agent_boot.trn_boot import _ntff_profile_via_ctypes
        mod = types.ModuleType('antenv.axon_hooks')
        hook = _ntff_profile_via_ctypes('/opt/axon/libaxon_pjrt.so')
        mod.get_axon_ntff_profile_hook = lambda: hook
        sys.modules['antenv.axon_hooks'] = mod
    except Exception:
        pass


_BUILT = None


def _build():
    global _BUILT
    if _BUILT is not None:
        return _BUILT
    _install_ntff_hook()
    import concourse.bacc as bacc
    import concourse.mybir as mybir
    from concourse import tile
    from concourse.masks import make_identity
    from contextlib import ExitStack

    F32 = mybir.dt.float32
    F32R = mybir.dt.float32r
    BF16 = mybir.dt.bfloat16
    AF = mybir.ActivationFunctionType
    ALU = mybir.AluOpType
    AX = mybir.AxisListType

    nc = bacc.Bacc("TRN2", target_bir_lowering=False, debug=False,
                   num_devices=NCORES)

    # ---- DRAM I/O (per core) ----
    d_xT = nc.dram_tensor("xT", [NIT, NDC, 128, S], BF16, kind="ExternalInput")
    d_xp = nc.dram_tensor("xp", [NIT, NSC, 128, D], F32, kind="ExternalInput")
    d_wqk = nc.dram_tensor("wqk", [2, 4, 128, 512], BF16, kind="ExternalInput")
    d_bqk = nc.dram_tensor("bqk", [128, 8], F32, kind="ExternalInput")
    d_wv = nc.dram_tensor("wv", [NDC, 128, 520], BF16, kind="ExternalInput")
    d_bvrow = nc.dram_tensor("bvrow", [128, 520], BF16, kind="ExternalInput")
    d_wo = nc.dram_tensor("wo", [4, 128, 512], BF16, kind="ExternalInput")
    d_w1 = nc.dram_tensor("w1", [NCD, NDC, 128, KS * 128], BF16,
                          kind="ExternalInput")
    d_w2 = nc.dram_tensor("w2", [NCD, 128, KS * 512], BF16,
                          kind="ExternalInput")
    d_bc1s = nc.dram_tensor("bc1s", [128, NCD], F32, kind="ExternalInput")
    d_gb = nc.dram_tensor("gb", [5, 128, 512], BF16, kind="ExternalInput")
    d_cones = nc.dram_tensor("cones", [128, 128], BF16, kind="ExternalInput")
    d_czero = nc.dram_tensor("czero", [128, 8], BF16, kind="ExternalInput")
    d_y = nc.dram_tensor("y", [NIT, NSC, 128, D], F32, kind="ExternalOutput")

    G1, B1, G2, B2, BC2 = range(5)

    with tile.TileContext(nc) as tc:
        est = ExitStack()
        with est:
            cp = est.enter_context(tc.tile_pool(name="const", bufs=1))
            pl = est.enter_context(tc.tile_pool(name="work", bufs=1))
            ps = est.enter_context(tc.tile_pool(name="psum", bufs=1, space="PSUM"))
            dp = est.enter_context(tc.tile_pool(name="dramp", bufs=1, space="DRAM"))

            h_dram = [[dp.tile([128, D], F32, tag=f"hd{it}_{sc}",
                               name=f"hd{it}_{sc}")
                       for sc in range(NSC)] for it in range(NIT)]

            # ---- constants ----
            t_bqk = cp.tile([128, 8], F32, tag="bqk")
            nc.sync.dma_start(t_bqk[:], d_bqk[:])
            t_gb = []
            for i in range(5):
                t = cp.tile([128, 512], BF16, tag=f"gb{i}", name=f"gb{i}")
                nc.sync.dma_start(t[:], d_gb[i])
                t_gb.append(t)
            t_bc1s = cp.tile([128, NCD], F32, tag="bc1s")
            nc.sync.dma_start(t_bc1s[:], d_bc1s[:])
            t_ident = cp.tile([128, 128], F32, tag="ident")
            make_identity(nc, t_ident[:])
            # PE warm-up: dense junk matmuls on the identity tile while the
            # first x/weight DMAs are in flight, so HAM reaches K=8/8 before
            # real work arrives.
            t_identb = cp.tile([128, 128], BF16, tag="identb")
            nc.vector.tensor_copy(t_identb[:], t_ident[:])
            pwu = ps.tile([128, 512], F32, tag="pp", bufs=3)
            for _ in range(48):
                nc.tensor.matmul(pwu[:, 0:128], t_identb[:], t_identb[:],
                                 start=True, stop=True)
            t_cones = cp.tile([128, 128], BF16, tag="cones")
            nc.sync.dma_start(t_cones[:], d_cones[:])
            t_czero = cp.tile([128, 8], BF16, tag="czero")
            nc.sync.dma_start(t_czero[:], d_czero[:])
            t_eps = cp.tile([128, 1], F32, tag="eps")
            nc.vector.memset(t_eps[:], EPS)
            t_wv = []
            for dc in range(NDC):
                t = cp.tile([128, 520], BF16, tag=f"wv{dc}", name=f"wv{dc}")
                nc.sync.dma_start(t[:], d_wv[dc])
                t_wv.append(t)
            t_bvrow = cp.tile([128, 520], BF16, tag="bvrow")
            nc.sync.dma_start(t_bvrow[:], d_bvrow[:])
            t_wo = []
            for c in range(4):
                t = cp.tile([128, 512], BF16, tag=f"wo{c}", name=f"wo{c}")
                nc.sync.dma_start(t[:], d_wo[c])
                t_wo.append(t)

            # persistent hT tiles (bf16, padded s)
            hT = [[pl.tile([128, S + 8], BF16, tag=f"ht{it}_{dc}",
                           name=f"ht{it}_{dc}")
                   for dc in range(NDC)] for it in range(NIT)]

            state = [dict() for _ in range(NIT)]

            # ================= emit helpers =================
            def emit_x(it):
                st = state[it]
                xt = []
                for dc in range(NDC):
                    t = pl.tile([128, S], BF16, tag=f"xt{dc}", name=f"xt{dc}")
                    nc.sync.dma_start(t[:], d_xT[it, dc])
                    xt.append(t)
                st["xt"] = xt
                st["qkt"] = {}

            def emit_v(it):
                """V projection for one item (dense PE block)."""
                st = state[it]
                xt = st["xt"]
                vst = []
                for tc_i in range(NSC):
                    vt = pl.tile([128, 520], BF16, tag=f"vst{tc_i}",
                                 name=f"vst{tc_i}")
                    for half in range(2):
                        colo = half * 260
                        pv = ps.tile([128, 260], F32, tag="pp", bufs=3)
                        for dc in range(NDC):
                            nc.tensor.matmul(
                                pv[:], xt[dc][:, tc_i * 128:(tc_i + 1) * 128],
                                t_wv[dc][:, colo:colo + 260],
                                start=(dc == 0), stop=False)
                        nc.tensor.matmul(
                            pv[:], t_cones[0:1, 0:128],
                            t_bvrow[0:1, colo:colo + 260],
                            start=False, stop=True)
                        nc.vector.tensor_copy(vt[:, colo:colo + 260], pv[:])
                    vst.append(vt)
                st["vst"] = vst

            def emit_qk(it, pair):
                st = state[it]
                xt = st["xt"]
                for proj in range(2):
                    wt = pl.tile([128, 512], BF16, tag=f"wqk{proj}",
                                 bufs=2, name="wt")
                    nc.sync.dma_start(wt[:], d_wqk[proj, pair])
                    qt = pl.tile([128, S], BF16, tag=f"qk{proj}{pair}",
                                 name="qt")
                    for scol in range(NCOL):
                        pq = ps.tile([128, 512], F32, tag="pp", bufs=3)
                        for dc in range(NDC):
                            nc.tensor.matmul(
                                pq[:], wt[:, dc * 128:(dc + 1) * 128],
                                xt[dc][:, scol * 512:(scol + 1) * 512],
                                start=(dc == 0), stop=(dc == NDC - 1))
                        nc.vector.tensor_scalar_add(
                            qt[:, scol * 512:(scol + 1) * 512], pq[:],
                            t_bqk[:, proj * 4 + pair:proj * 4 + pair + 1])
                    st["qkt"][(proj, pair)] = qt

            def emit_heads_pair(it, pair):
                st = state[it]
                if pair == 0:
                    st["ctxT"] = [pl.tile([128, S], BF16, tag=f"ct{c}",
                                          name=f"ct{c}") for c in range(4)]
                qT = st["qkt"][(0, pair)]
                kT = st["qkt"][(1, pair)]
                vst = st["vst"]
                ctxT = st["ctxT"]
                for sub in range(2):
                    h = 2 * pair + sub
                    hr = slice(sub * 64, sub * 64 + 64)
                    for scol in range(NCOL):
                        so = scol * 512
                        pex = []
                        for ti in range(NSC):
                            pp = ps.tile([128, 512], F32, tag="pp", bufs=3)
                            nc.tensor.matmul(
                                pp[:], kT[hr, ti * 128:(ti + 1) * 128],
                                qT[hr, so:so + 512], start=True, stop=True)
                            pe = pl.tile([128, 512], BF16, tag=f"pex{ti}",
                                         bufs=1, name="pe")
                            nc.scalar.activation(pe[:], pp[:], AF.Exp,
                                                 scale=0.125)
                            pex.append(pe)
                        pc = ps.tile([65, 512], F32, tag="pc", bufs=1)
                        for ti in range(NSC):
                            nc.tensor.matmul(
                                pc[:], vst[ti][:, h * 65:h * 65 + 65],
                                pex[ti][:], start=(ti == 0),
                                stop=(ti == NSC - 1))
                        zr = pl.tile([64, 512], BF16, tag="zrt", bufs=2,
                                     name="zr")
                        nc.vector.tensor_copy(zr[0:1, :], pc[64:65, :])
                        pb = ps.tile([64, 512], F32, tag="pp", bufs=3)
                        nc.tensor.matmul(pb[:], t_cones[0:1, 0:64], zr[0:1, :],
                                         start=True, stop=True)
                        bcs = pl.tile([64, 512], F32, tag="bcs", bufs=2,
                                      name="bcs")
                        nc.vector.reciprocal_approx_fast(out=bcs[:], in_=pb[:])
                        nc.vector.tensor_tensor(
                            ctxT[pair][hr, so:so + 512], pc[0:64, :],
                            bcs[:], ALU.mult)


            def emit_tail(it):
                """Wo + residual + LN1 + transpose into hT (+ h spill).
                Fully per-sc (LN rows are independent s positions)."""
                st = state[it]
                ctxT = st["ctxT"]
                for sc in range(NSC):
                    xpt = pl.tile([128, 512], F32, tag="xpt", bufs=2)
                    nc.sync.dma_start(xpt[:], d_xp[it, sc])
                    pw = ps.tile([128, 512], F32, tag="pc", bufs=1)
                    for c in range(4):
                        nc.tensor.matmul(
                            pw[:], ctxT[c][:, sc * 128:(sc + 1) * 128],
                            t_wo[c][:], start=(c == 0), stop=(c == 3))
                    r = pl.tile([128, 512], F32, tag="res_t", bufs=2, name="r")
                    nc.vector.tensor_tensor(r[:], pw[:], xpt[:], ALU.add)
                    st1 = pl.tile([128, 2], F32, tag="st1", bufs=3)
                    nc.vector.reduce_sum(st1[:, 0:1], r[:], axis=AX.X)
                    sq = pl.tile([128, 512], BF16, tag="sqs", bufs=2, name="sq")
                    nc.scalar.activation(sq[:], r[:], AF.Square,
                                         accum_out=st1[:, 1:2])
                    mv = pl.tile([128, 2], F32, tag="mv1", bufs=3)
                    nc.vector.tensor_scalar_mul(mv[:], st1[:], 1.0 / D)
                    inv1 = pl.tile([128, 1], F32, tag="inv1", bufs=3)
                    nc.vector.tensor_tensor(inv1[:], mv[:, 0:1], mv[:, 0:1],
                                            ALU.mult)
                    nc.vector.tensor_tensor(inv1[:], mv[:, 1:2], inv1[:],
                                            ALU.subtract)
                    nc.scalar.activation(inv1[:], inv1[:], AF.Sqrt,
                                         bias=t_eps[:])
                    nc.vector.reciprocal(inv1[:], inv1[:])
                    ht_ = pl.tile([128, 512], F32, tag="hst", bufs=2, name="h_")
                    nc.vector.tensor_scalar(
                        ht_[:], r[:], mv[:, 0:1], inv1[:, 0:1],
                        ALU.subtract, ALU.mult)
                    nc.vector.tensor_tensor(ht_[:], ht_[:], t_gb[G1][:],
                                            ALU.mult)
                    nc.vector.tensor_tensor(ht_[:], ht_[:], t_gb[B1][:],
                                            ALU.add)
                    nc.sync.dma_start(h_dram[it][sc][:], ht_[:])
                    for dc in range(NDC):
                        pt = ps.tile([128, 128], F32, tag="pp", bufs=3)
                        nc.tensor.transpose(pt[:], ht_[:, dc * 128:(dc + 1) * 128],
                                            t_ident[:])
                        nc.scalar.copy(
                            hT[it][dc][:, 4 + sc * 128: 4 + (sc + 1) * 128],
                            pt[:])
                for dc in range(NDC):
                    nc.sync.dma_start(hT[it][dc][:, 0:4], d_czero[:, 0:4])
                    nc.sync.dma_start(hT[it][dc][:, S + 4:S + 8],
                                      d_czero[:, 4:8])

            o2 = [[None] * NSC for _ in range(NIT)]
            GSZ = 4                       # cdc chunks per conv2 group
            NG = NCD // GSZ               # 4 groups

            def emit_conv1_chunk(it, cdc, slot):
                """conv1 for one cdc chunk -> c1t tile (slot 0..3 in group)."""
                w1t = []
                for dc in range(NDC):
                    t = pl.tile([128, KS * 128], BF16, tag=f"w1t{dc}", bufs=2,
                                name="w1t")
                    nc.sync.dma_start(t[:], d_w1[cdc, dc])
                    w1t.append(t)
                c1t = pl.tile([128, S + 8], BF16, tag=f"c1t{slot}", bufs=2,
                              name="c1t")
                nc.sync.dma_start(c1t[:, 0:4], d_czero[:, 0:4])
                nc.sync.dma_start(c1t[:, S + 4:S + 8], d_czero[:, 4:8])
                for scol in range(NCOL):
                    pc1 = ps.tile([128, 512], F32, tag="c1p", bufs=2)
                    idx = 0
                    for k in range(KS):
                        for dc in range(NDC):
                            nc.tensor.matmul(
                                pc1[:], w1t[dc][:, k * 128:(k + 1) * 128],
                                hT[it][dc][:, scol * 512 + k:
                                           scol * 512 + k + 512],
                                start=(idx == 0), stop=(idx == 35))
                            idx += 1
                    nc.scalar.activation(
                        c1t[:, 4 + scol * 512: 4 + (scol + 1) * 512],
                        pc1[:], AF.Relu, bias=t_bc1s[:, cdc:cdc + 1])
                return c1t

            def emit_ln2_sc(it, sc):
                """Per-s-chunk LN2: fully independent per row -> no batching."""
                t1 = pl.tile([128, 512], F32, tag="hst", bufs=2)
                nc.vector.tensor_tensor(t1[:], o2[it][sc][:], t_gb[BC2][:],
                                        ALU.add)
                nc.scalar.activation(t1[:], t1[:], AF.Relu)
                hrl = pl.tile([128, 512], F32, tag="xpt", bufs=2)
                nc.sync.dma_start(hrl[:], h_dram[it][sc][:])
                r = pl.tile([128, 512], F32, tag="res_ln2", bufs=2, name="r2")
                nc.vector.tensor_tensor(r[:], t1[:], hrl[:], ALU.add)
                st1 = pl.tile([128, 2], F32, tag="st1", bufs=3)
                nc.vector.reduce_sum(st1[:, 0:1], r[:], axis=AX.X)
                sq = pl.tile([128, 512], BF16, tag="sqs", bufs=2, name="sq2")
                nc.scalar.activation(sq[:], r[:], AF.Square,
                                     accum_out=st1[:, 1:2])
                mv = pl.tile([128, 2], F32, tag="mv1", bufs=3)
                nc.vector.tensor_scalar_mul(mv[:], st1[:], 1.0 / D)
                inv1 = pl.tile([128, 1], F32, tag="inv1", bufs=3)
                nc.vector.tensor_tensor(inv1[:], mv[:, 0:1], mv[:, 0:1],
                                        ALU.mult)
                nc.vector.tensor_tensor(inv1[:], mv[:, 1:2], inv1[:],
                                        ALU.subtract)
                nc.scalar.activation(inv1[:], inv1[:], AF.Sqrt, bias=t_eps[:])
                nc.vector.reciprocal(inv1[:], inv1[:])
                yt = pl.tile([128, 512], F32, tag="hst", bufs=2)
                nc.vector.tensor_scalar(
                    yt[:], r[:], mv[:, 0:1], inv1[:, 0:1],
                    ALU.subtract, ALU.mult)
                nc.vector.tensor_tensor(yt[:], yt[:], t_gb[G2][:], ALU.mult)
                nc.vector.tensor_tensor(yt[:], yt[:], t_gb[B2][:], ALU.add)
                nc.sync.dma_start(d_y[it, sc], yt[:])

            def emit_conv2_group(it, g, c1ts, last):
                """conv2 accumulated over a 4-cdc group in PSUM; on the last
                group, fuse per-sc LN2 right after each sc completes."""
                w2t = []
                for j in range(GSZ):
                    t = pl.tile([128, KS * 512], BF16, tag=f"w2t{j}", bufs=1,
                                name="w2t")
                    nc.sync.dma_start(t[:], d_w2[g * GSZ + j])
                    w2t.append(t)
                for sc in range(NSC):
                    pc2 = ps.tile([128, 512], F32, tag="c2p", bufs=2)
                    idx = 0
                    for j in range(GSZ):
                        for k in range(KS):
                            nc.tensor.matmul(
                                pc2[:],
                                c1ts[j][:, sc * 128 + k: sc * 128 + k + 128],
                                w2t[j][:, k * 512:(k + 1) * 512],
                                start=(idx == 0), stop=(idx == GSZ * KS - 1))
                            idx += 1
                    if g == 0:
                        t = pl.tile([128, 512], F32, tag=f"o2_{sc}",
                                    bufs=1, name=f"o2_{sc}")
                        o2[it][sc] = t
                        nc.vector.tensor_copy(t[:], pc2[:])
                    else:
                        nc.vector.tensor_tensor(o2[it][sc][:], pc2[:],
                                                o2[it][sc][:], ALU.add)
                    if last:
                        emit_ln2_sc(it, sc)

            # ================= emission order =================
            emit_x(0)
            emit_v(0)
            for pair in range(4):
                emit_qk(0, pair)
            emit_x(1)
            for pair in range(4):
                emit_heads_pair(0, pair)
                emit_qk(1, pair)
            emit_v(1)
            emit_tail(0)
            for g in range(NG):
                c1ts = [emit_conv1_chunk(0, g * GSZ + j, j)
                        for j in range(GSZ)]
                if g < 2:
                    emit_heads_pair(1, 2 * g)
                    emit_heads_pair(1, 2 * g + 1)
                emit_conv2_group(0, g, c1ts, last=(g == NG - 1))
                if g == 1:
                    emit_tail(1)
            for g in range(NG):
                c1ts = [emit_conv1_chunk(1, g * GSZ + j, j)
                        for j in range(GSZ)]
                emit_conv2_group(1, g, c1ts, last=(g == NG - 1))

    nc.compile()
    _BUILT = nc
    return nc


def _prep_host(inputs):
    import ml_dtypes
    bf16 = ml_dtypes.bfloat16
    x = np.asarray(inputs["x"], np.float32)
    Wq = np.asarray(inputs["Wq"], np.float32)
    bq = np.asarray(inputs["bq"], np.float32)
    Wk = np.asarray(inputs["Wk"], np.float32)
    bk = np.asarray(inputs["bk"], np.float32)
    Wv = np.asarray(inputs["Wv"], np.float32)
    bv = np.asarray(inputs["bv"], np.float32)
    Wo = np.asarray(inputs["Wo"], np.float32)
    bo = np.asarray(inputs["bo"], np.float32)
    g1 = np.asarray(inputs["g1"], np.float32)
    b1 = np.asarray(inputs["b1"], np.float32)
    g2 = np.asarray(inputs["g2"], np.float32)
    b2 = np.asarray(inputs["b2"], np.float32)
    Wc1 = np.asarray(inputs["Wc1"], np.float32)
    bc1 = np.asarray(inputs["bc1"], np.float32)
    Wc2 = np.asarray(inputs["Wc2"], np.float32)
    bc2 = np.asarray(inputs["bc2"], np.float32)

    xT = np.ascontiguousarray(x.transpose(0, 2, 1).reshape(B, NDC, 128, S)) \
           .astype(bf16)
    xp = np.ascontiguousarray((x + bo[None, None, :]).reshape(B, NSC, 128, D))

    wqk = np.zeros((2, 4, 128, 512), np.float32)
    for proj, W in ((0, Wq), (1, Wk)):
        for pair in range(4):
            blk = np.concatenate([W[2 * pair], W[2 * pair + 1]], axis=1)
            wqk[proj, pair] = blk.reshape(NDC, 128, 128).transpose(1, 0, 2) \
                                 .reshape(128, 512)
    bqk = np.zeros((128, 8), np.float32)
    for proj, b in ((0, bq), (1, bk)):
        for pair in range(4):
            bqk[:, proj * 4 + pair] = np.concatenate(
                [b[2 * pair], b[2 * pair + 1]])

    wv = np.zeros((NDC, 128, 520), np.float32)
    bvrow = np.zeros((128, 520), np.float32)
    for h in range(H):
        wv[:, :, h * 65:h * 65 + 64] = Wv[h].reshape(NDC, 128, 64)
        bvrow[0, h * 65:h * 65 + 64] = bv[h]
        bvrow[0, h * 65 + 64] = 1.0

    wo = np.ascontiguousarray(Wo.reshape(4, 128, 512))

    w1 = np.ascontiguousarray(
        Wc1.reshape(NCD, 128, NDC, 128, KS).transpose(0, 2, 3, 4, 1)
           .reshape(NCD, NDC, 128, KS * 128)).astype(bf16)
    w2 = np.ascontiguousarray(
        Wc2.reshape(D, NCD, 128, KS).transpose(1, 2, 3, 0)
           .reshape(NCD, 128, KS * 512)).astype(bf16)
    bc1s = np.ascontiguousarray(bc1.reshape(NCD, 128).T)

    gb = np.stack([np.tile(v[None, :], (128, 1))
                   for v in (g1, b1, g2, b2, bc2)]).astype(bf16)
    cones = np.ones((128, 128), bf16)
    czero = np.zeros((128, 8), bf16)

    shared = dict(wqk=wqk.astype(bf16), bqk=bqk, wv=wv.astype(bf16),
                  bvrow=bvrow.astype(bf16), wo=wo.astype(bf16),
                  w1=w1, w2=w2, bc1s=bc1s, gb=gb, cones=cones, czero=czero)
    in_maps = []
    for c in range(NCORES):
        m = dict(shared)
        m["xT"] = np.ascontiguousarray(xT[c * NIT:(c + 1) * NIT])
        m["xp"] = np.ascontiguousarray(xp[c * NIT:(c + 1) * NIT])
        in_maps.append(m)
    return in_maps


def run(inputs, trace=False, **trace_kwargs):
    nc = _build()
    from concourse.bass_utils import run_bass_kernel_spmd
    in_maps = _prep_host(inputs)
    res = run_bass_kernel_spmd(nc, in_maps, core_ids=list(range(NCORES)),
                               trace=trace, **trace_kwargs)
    y = np.concatenate([res.results[c]["y"].reshape(NIT, S, D)
                        for c in range(NCORES)], axis=0)
    return y, res


def kernel(**inputs):
    y, _ = run(inputs, trace=False)
    return y



# revision 23
# speedup vs baseline: 1.0457x; 1.0121x over previous
"""Trainium2 Bass kernel for the FFT-block (attention + conv FFN) problem.

Sharding: data-parallel over batch. B=16 items across 8 cores -> 2 items/core.
Each core runs the full block for its items; no collectives.

Per item:
  - attention via scores^T = K Q^T (softmax sums land on the partition axis and
    are folded into the ctx matmul through a ones-column appended to V); the
    per-head 1/Z normalization is broadcast across partitions with a K=1 PE
    matmul.  Attention matmuls run in fp32r (tf32-like, fp32 accumulate);
    softmax weights and V are bf16.
  - convs are 9 shifted matmuls over transposed activations hT [D, S_pad] in
    bf16 (weights+activations), fp32 PSUM accumulation and fp32 o2 accumulator.
  - emission order software-pipelines item1's attention into item0's conv
    stream so the PE never drains (HAM stays at K=8/8).
"""
import sys, types
import numpy as np

B, S, D = 16, 1024, 512
H, DK = 8, 64
CD, KS = 2048, 9
EPS = 1e-5
NCORES = 8
NIT = B // NCORES
NDC = D // 128             # 4 d-chunks
NSC = S // 128             # 8 s-chunks
NCOL = S // 512            # 2 s-cols
NCD = CD // 128            # 16 cd-chunks


def _install_ntff_hook():
    try:
        from antenv.axon_hooks import get_axon_ntff_profile_hook  # noqa
        return
    except ImportError:
        pass
    try:
        from trn_agent_boot.trn_boot import _ntff_profile_via_ctypes
        mod = types.ModuleType('antenv.axon_hooks')
        hook = _ntff_profile_via_ctypes('/opt/axon/libaxon_pjrt.so')
        mod.get_axon_ntff_profile_hook = lambda: hook
        sys.modules['antenv.axon_hooks'] = mod
    except Exception:
        pass


_BUILT = None


def _build():
    global _BUILT
    if _BUILT is not None:
        return _BUILT
    _install_ntff_hook()
    import concourse.bacc as bacc
    import concourse.mybir as mybir
    from concourse import tile
    from concourse.masks import make_identity
    from contextlib import ExitStack

    F32 = mybir.dt.float32
    F32R = mybir.dt.float32r
    BF16 = mybir.dt.bfloat16
    AF = mybir.ActivationFunctionType
    ALU = mybir.AluOpType
    AX = mybir.AxisListType

    nc = bacc.Bacc("TRN2", target_bir_lowering=False, debug=False,
                   num_devices=NCORES)

    # ---- DRAM I/O (per core) ----
    d_xT = nc.dram_tensor("xT", [NIT, NDC, 128, S], BF16, kind="ExternalInput")
    d_xp = nc.dram_tensor("xp", [NIT, NSC, 128, D], F32, kind="ExternalInput")
    d_wqk = nc.dram_tensor("wqk", [2, 4, 128, 512], BF16, kind="ExternalInput")
    d_bqk = nc.dram_tensor("bqk", [128, 8], F32, kind="ExternalInput")
    d_wv = nc.dram_tensor("wv", [NDC, 128, 520], BF16, kind="ExternalInput")
    d_bvrow = nc.dram_tensor("bvrow", [128, 520], BF16, kind="ExternalInput")
    d_wo = nc.dram_tensor("wo", [4, 128, 512], BF16, kind="ExternalInput")
    d_w1 = nc.dram_tensor("w1", [NCD, NDC, 128, KS * 128], BF16,
                          kind="ExternalInput")
    d_w2 = nc.dram_tensor("w2", [NCD, 128, KS * 512], BF16,
                          kind="ExternalInput")
    d_bc1s = nc.dram_tensor("bc1s", [128, NCD], F32, kind="ExternalInput")
    d_gb = nc.dram_tensor("gb", [5, 128, 512], BF16, kind="ExternalInput")
    d_cones = nc.dram_tensor("cones", [128, 128], BF16, kind="ExternalInput")
    d_czero = nc.dram_tensor("czero", [128, 8], BF16, kind="ExternalInput")
    d_y = nc.dram_tensor("y", [NIT, NSC, 128, D], F32, kind="ExternalOutput")

    G1, B1, G2, B2, BC2 = range(5)

    with tile.TileContext(nc) as tc:
        est = ExitStack()
        with est:
            cp = est.enter_context(tc.tile_pool(name="const", bufs=1))
            pl = est.enter_context(tc.tile_pool(name="work", bufs=1))
            ps = est.enter_context(tc.tile_pool(name="psum", bufs=1, space="PSUM"))
            dp = est.enter_context(tc.tile_pool(name="dramp", bufs=1, space="DRAM"))

            h_dram = [[dp.tile([128, D], F32, tag=f"hd{it}_{sc}",
                               name=f"hd{it}_{sc}")
                       for sc in range(NSC)] for it in range(NIT)]

            # ---- PE warm-up + prioritized x DMA ----
            # ident needs no DMA; junk matmuls keep the PE busy (HAM K=8/8)
            # while the first x/weight DMAs land.
            t_ident = cp.tile([128, 128], F32, tag="ident")
            make_identity(nc, t_ident[:])
            t_identb = cp.tile([128, 128], BF16, tag="identb")
            nc.vector.tensor_copy(t_identb[:], t_ident[:])
            # x DMA for item 0 queued before all const DMAs
            xt0 = []
            for dc in range(NDC):
                t = pl.tile([128, S], BF16, tag=f"xt{dc}", name=f"xt{dc}")
                nc.sync.dma_start(t[:], d_xT[0, dc])
                xt0.append(t)
            pwu = ps.tile([128, 512], F32, tag="pp", bufs=3)
            for _ in range(100):
                nc.tensor.matmul(pwu[:, 0:128], t_identb[:], t_identb[:],
                                 start=True, stop=True)

            # ---- constants ----
            t_bqk = cp.tile([128, 8], F32, tag="bqk")
            nc.sync.dma_start(t_bqk[:], d_bqk[:])
            t_gb = []
            for i in range(5):
                t = cp.tile([128, 512], BF16, tag=f"gb{i}", name=f"gb{i}")
                nc.sync.dma_start(t[:], d_gb[i])
                t_gb.append(t)
            t_bc1s = cp.tile([128, NCD], F32, tag="bc1s")
            nc.sync.dma_start(t_bc1s[:], d_bc1s[:])
            t_cones = cp.tile([128, 128], BF16, tag="cones")
            nc.sync.dma_start(t_cones[:], d_cones[:])
            t_czero = cp.tile([128, 8], BF16, tag="czero")
            nc.sync.dma_start(t_czero[:], d_czero[:])
            t_eps = cp.tile([128, 1], F32, tag="eps")
            nc.vector.memset(t_eps[:], EPS)
            t_wv = []
            for dc in range(NDC):
                t = cp.tile([128, 520], BF16, tag=f"wv{dc}", name=f"wv{dc}")
                nc.sync.dma_start(t[:], d_wv[dc])
                t_wv.append(t)
            t_bvrow = cp.tile([128, 520], BF16, tag="bvrow")
            nc.sync.dma_start(t_bvrow[:], d_bvrow[:])
            t_wo = []
            for c in range(4):
                t = cp.tile([128, 512], BF16, tag=f"wo{c}", name=f"wo{c}")
                nc.sync.dma_start(t[:], d_wo[c])
                t_wo.append(t)

            # persistent hT tiles (bf16, padded s)
            hT = [[pl.tile([128, S + 8], BF16, tag=f"ht{it}_{dc}",
                           name=f"ht{it}_{dc}")
                   for dc in range(NDC)] for it in range(NIT)]

            state = [dict() for _ in range(NIT)]

            # ================= emit helpers =================
            def emit_x(it):
                st = state[it]
                if it == 0:
                    st["xt"] = xt0
                else:
                    xt = []
                    for dc in range(NDC):
                        t = pl.tile([128, S], BF16, tag=f"xt{dc}",
                                    name=f"xt{dc}")
                        nc.sync.dma_start(t[:], d_xT[it, dc])
                        xt.append(t)
                    st["xt"] = xt
                st["qkt"] = {}

            def emit_v(it):
                """V projection for one item (dense PE block)."""
                st = state[it]
                xt = st["xt"]
                vst = []
                for tc_i in range(NSC):
                    vt = pl.tile([128, 520], BF16, tag=f"vst{tc_i}",
                                 name=f"vst{tc_i}")
                    for half in range(2):
                        colo = half * 260
                        pv = ps.tile([128, 260], F32, tag="pp", bufs=3)
                        for dc in range(NDC):
                            nc.tensor.matmul(
                                pv[:], xt[dc][:, tc_i * 128:(tc_i + 1) * 128],
                                t_wv[dc][:, colo:colo + 260],
                                start=(dc == 0), stop=False)
                        nc.tensor.matmul(
                            pv[:], t_cones[0:1, 0:128],
                            t_bvrow[0:1, colo:colo + 260],
                            start=False, stop=True)
                        nc.vector.tensor_copy(vt[:, colo:colo + 260], pv[:])
                    vst.append(vt)
                st["vst"] = vst

            def emit_qk(it, pair):
                st = state[it]
                xt = st["xt"]
                for proj in range(2):
                    wt = pl.tile([128, 512], BF16, tag=f"wqk{proj}",
                                 bufs=2, name="wt")
                    nc.sync.dma_start(wt[:], d_wqk[proj, pair])
                    qt = pl.tile([128, S], BF16, tag=f"qk{proj}{pair}",
                                 name="qt")
                    for scol in range(NCOL):
                        pq = ps.tile([128, 512], F32, tag="pp", bufs=3)
                        for dc in range(NDC):
                            nc.tensor.matmul(
                                pq[:], wt[:, dc * 128:(dc + 1) * 128],
                                xt[dc][:, scol * 512:(scol + 1) * 512],
                                start=(dc == 0), stop=(dc == NDC - 1))
                        nc.vector.tensor_scalar_add(
                            qt[:, scol * 512:(scol + 1) * 512], pq[:],
                            t_bqk[:, proj * 4 + pair:proj * 4 + pair + 1])
                    st["qkt"][(proj, pair)] = qt

            def emit_heads_pair(it, pair):
                """Both heads of the pair together: scores for the two heads
                run concurrently via PE row-tiling (K=64 sub-arrays); all 16
                exp tiles of a scol are in flight before ctx consumes them;
                the ctx PSUM bank is drained with fast copies so the next
                accumulation can start immediately."""
                st = state[it]
                if pair == 0:
                    st["ctxT"] = [pl.tile([128, S], BF16, tag=f"ct{c}",
                                          name=f"ct{c}") for c in range(4)]
                qT = st["qkt"][(0, pair)]
                kT = st["qkt"][(1, pair)]
                vst = st["vst"]
                ctxT = st["ctxT"]
                for scol in range(NCOL):
                    so = scol * 512
                    pex = {}
                    for ti in range(NSC):
                        for sub in range(2):
                            hr = slice(sub * 64, sub * 64 + 64)
                            pp = ps.tile([128, 512], F32, tag="pp", bufs=3)
                            nc.tensor.matmul(
                                pp[:], kT[hr, ti * 128:(ti + 1) * 128],
                                qT[hr, so:so + 512], start=True, stop=True)
                            pe = pl.tile([128, 512], BF16,
                                         tag=f"pex{sub}{ti}", bufs=1,
                                         name="pe")
                            nc.scalar.activation(pe[:], pp[:], AF.Exp,
                                                 scale=0.125)
                            pex[(sub, ti)] = pe
                    for sub in range(2):
                        h = 2 * pair + sub
                        hr = slice(sub * 64, sub * 64 + 64)
                        pc = ps.tile([65, 512], F32, tag="pc", bufs=1)
                        for ti in range(NSC):
                            nc.tensor.matmul(
                                pc[:], vst[ti][:, h * 65:h * 65 + 65],
                                pex[(sub, ti)][:], start=(ti == 0),
                                stop=(ti == NSC - 1))
                        zr = pl.tile([64, 512], BF16, tag="zrt", bufs=2,
                                     name="zr")
                        nc.vector.tensor_copy(zr[0:1, :], pc[64:65, :])
                        craw = pl.tile([64, 512], BF16, tag="craw", bufs=2,
                                       name="craw")
                        nc.vector.tensor_copy(craw[:], pc[0:64, :])
                        pb = ps.tile([64, 512], F32, tag="pp", bufs=3)
                        nc.tensor.matmul(pb[:], t_cones[0:1, 0:64], zr[0:1, :],
                                         start=True, stop=True)
                        bcs = pl.tile([64, 512], F32, tag="bcs", bufs=2,
                                      name="bcs")
                        nc.vector.reciprocal_approx_fast(out=bcs[:], in_=pb[:])
                        nc.vector.tensor_tensor(
                            ctxT[pair][hr, so:so + 512], craw[:],
                            bcs[:], ALU.mult)


            def emit_tail(it):
                """Wo + residual + LN1 + transpose into hT (+ h spill).
                Fully per-sc (LN rows are independent s positions)."""
                st = state[it]
                ctxT = st["ctxT"]
                for sc in range(NSC):
                    xpt = pl.tile([128, 512], F32, tag="xpt", bufs=2)
                    nc.sync.dma_start(xpt[:], d_xp[it, sc])
                    pw = ps.tile([128, 512], F32, tag="pc", bufs=1)
                    for c in range(4):
                        nc.tensor.matmul(
                            pw[:], ctxT[c][:, sc * 128:(sc + 1) * 128],
                            t_wo[c][:], start=(c == 0), stop=(c == 3))
                    r = pl.tile([128, 512], F32, tag="res_t", bufs=2, name="r")
                    nc.vector.tensor_tensor(r[:], pw[:], xpt[:], ALU.add)
                    st1 = pl.tile([128, 2], F32, tag="st1", bufs=3)
                    nc.vector.reduce_sum(st1[:, 0:1], r[:], axis=AX.X)
                    sq = pl.tile([128, 512], BF16, tag="sqs", bufs=2, name="sq")
                    nc.scalar.activation(sq[:], r[:], AF.Square,
                                         accum_out=st1[:, 1:2])
                    mv = pl.tile([128, 2], F32, tag="mv1", bufs=3)
                    nc.vector.tensor_scalar_mul(mv[:], st1[:], 1.0 / D)
                    inv1 = pl.tile([128, 1], F32, tag="inv1", bufs=3)
                    nc.vector.tensor_tensor(inv1[:], mv[:, 0:1], mv[:, 0:1],
                                            ALU.mult)
                    nc.vector.tensor_tensor(inv1[:], mv[:, 1:2], inv1[:],
                                            ALU.subtract)
                    nc.scalar.activation(inv1[:], inv1[:], AF.Sqrt,
                                         bias=t_eps[:])
                    nc.vector.reciprocal(inv1[:], inv1[:])
                    ht_ = pl.tile([128, 512], F32, tag="hst", bufs=2, name="h_")
                    nc.vector.tensor_scalar(
                        ht_[:], r[:], mv[:, 0:1], inv1[:, 0:1],
                        ALU.subtract, ALU.mult)
                    nc.vector.tensor_tensor(ht_[:], ht_[:], t_gb[G1][:],
                                            ALU.mult)
                    nc.vector.tensor_tensor(ht_[:], ht_[:], t_gb[B1][:],
                                            ALU.add)
                    nc.sync.dma_start(h_dram[it][sc][:], ht_[:])
                    for dc in range(NDC):
                        pt = ps.tile([128, 128], F32, tag="pp", bufs=3)
                        nc.tensor.transpose(pt[:], ht_[:, dc * 128:(dc + 1) * 128],
                                            t_ident[:])
                        nc.scalar.copy(
                            hT[it][dc][:, 4 + sc * 128: 4 + (sc + 1) * 128],
                            pt[:])
                for dc in range(NDC):
                    nc.sync.dma_start(hT[it][dc][:, 0:4], d_czero[:, 0:4])
                    nc.sync.dma_start(hT[it][dc][:, S + 4:S + 8],
                                      d_czero[:, 4:8])

            o2 = [[None] * NSC for _ in range(NIT)]
            GSZ = 4                       # cdc chunks per conv2 group
            NG = NCD // GSZ               # 4 groups

            def emit_conv1_chunk(it, cdc, slot):
                """conv1 for one cdc chunk -> c1t tile (slot 0..3 in group)."""
                w1t = []
                for dc in range(NDC):
                    t = pl.tile([128, KS * 128], BF16, tag=f"w1t{dc}", bufs=2,
                                name="w1t")
                    nc.sync.dma_start(t[:], d_w1[cdc, dc])
                    w1t.append(t)
                c1t = pl.tile([128, S + 8], BF16, tag=f"c1t{slot}", bufs=2,
                              name="c1t")
                nc.sync.dma_start(c1t[:, 0:4], d_czero[:, 0:4])
                nc.sync.dma_start(c1t[:, S + 4:S + 8], d_czero[:, 4:8])
                for scol in range(NCOL):
                    pc1 = ps.tile([128, 512], F32, tag="c1p", bufs=2)
                    idx = 0
                    for k in range(KS):
                        for dc in range(NDC):
                            nc.tensor.matmul(
                                pc1[:], w1t[dc][:, k * 128:(k + 1) * 128],
                                hT[it][dc][:, scol * 512 + k:
                                           scol * 512 + k + 512],
                                start=(idx == 0), stop=(idx == 35))
                            idx += 1
                    nc.scalar.activation(
                        c1t[:, 4 + scol * 512: 4 + (scol + 1) * 512],
                        pc1[:], AF.Relu, bias=t_bc1s[:, cdc:cdc + 1])
                return c1t

            def emit_ln2_sc(it, sc):
                """Per-s-chunk LN2: fully independent per row -> no batching."""
                t1 = pl.tile([128, 512], F32, tag="hst", bufs=2)
                nc.vector.tensor_tensor(t1[:], o2[it][sc][:], t_gb[BC2][:],
                                        ALU.add)
                nc.scalar.activation(t1[:], t1[:], AF.Relu)
                hrl = pl.tile([128, 512], F32, tag="xpt", bufs=2)
                nc.sync.dma_start(hrl[:], h_dram[it][sc][:])
                r = pl.tile([128, 512], F32, tag="res_ln2", bufs=2, name="r2")
                nc.vector.tensor_tensor(r[:], t1[:], hrl[:], ALU.add)
                st1 = pl.tile([128, 2], F32, tag="st1", bufs=3)
                nc.vector.reduce_sum(st1[:, 0:1], r[:], axis=AX.X)
                sq = pl.tile([128, 512], BF16, tag="sqs", bufs=2, name="sq2")
                nc.scalar.activation(sq[:], r[:], AF.Square,
                                     accum_out=st1[:, 1:2])
                mv = pl.tile([128, 2], F32, tag="mv1", bufs=3)
                nc.vector.tensor_scalar_mul(mv[:], st1[:], 1.0 / D)
                inv1 = pl.tile([128, 1], F32, tag="inv1", bufs=3)
                nc.vector.tensor_tensor(inv1[:], mv[:, 0:1], mv[:, 0:1],
                                        ALU.mult)
                nc.vector.tensor_tensor(inv1[:], mv[:, 1:2], inv1[:],
                                        ALU.subtract)
                nc.scalar.activation(inv1[:], inv1[:], AF.Sqrt, bias=t_eps[:])
                nc.vector.reciprocal(inv1[:], inv1[:])
                yt = pl.tile([128, 512], F32, tag="hst", bufs=2)
                nc.vector.tensor_scalar(
                    yt[:], r[:], mv[:, 0:1], inv1[:, 0:1],
                    ALU.subtract, ALU.mult)
                nc.vector.tensor_tensor(yt[:], yt[:], t_gb[G2][:], ALU.mult)
                nc.vector.tensor_tensor(yt[:], yt[:], t_gb[B2][:], ALU.add)
                nc.sync.dma_start(d_y[it, sc], yt[:])

            def emit_conv2_group(it, g, c1ts, last):
                """conv2 accumulated over a 4-cdc group in PSUM; on the last
                group, fuse per-sc LN2 right after each sc completes."""
                w2t = []
                for j in range(GSZ):
                    t = pl.tile([128, KS * 512], BF16, tag=f"w2t{j}", bufs=1,
                                name="w2t")
                    nc.sync.dma_start(t[:], d_w2[g * GSZ + j])
                    w2t.append(t)
                for sc in range(NSC):
                    pc2 = ps.tile([128, 512], F32, tag="c2p", bufs=2)
                    idx = 0
                    for j in range(GSZ):
                        for k in range(KS):
                            nc.tensor.matmul(
                                pc2[:],
                                c1ts[j][:, sc * 128 + k: sc * 128 + k + 128],
                                w2t[j][:, k * 512:(k + 1) * 512],
                                start=(idx == 0), stop=(idx == GSZ * KS - 1))
                            idx += 1
                    if g == 0:
                        t = pl.tile([128, 512], F32, tag=f"o2_{sc}",
                                    bufs=1, name=f"o2_{sc}")
                        o2[it][sc] = t
                        nc.vector.tensor_copy(t[:], pc2[:])
                    else:
                        nc.vector.tensor_tensor(o2[it][sc][:], pc2[:],
                                                o2[it][sc][:], ALU.add)
                    if last:
                        emit_ln2_sc(it, sc)

            # ================= emission order =================
            emit_x(0)
            emit_v(0)
            for pair in range(4):
                emit_qk(0, pair)
            emit_x(1)
            for pair in range(4):
                emit_heads_pair(0, pair)
                emit_qk(1, pair)
            emit_v(1)
            emit_tail(0)
            for g in range(NG):
                c1ts = [emit_conv1_chunk(0, g * GSZ + j, j)
                        for j in range(GSZ)]
                if g < 2:
                    emit_heads_pair(1, 2 * g)
                    emit_heads_pair(1, 2 * g + 1)
                emit_conv2_group(0, g, c1ts, last=(g == NG - 1))
                if g == 1:
                    emit_tail(1)
            for g in range(NG):
                c1ts = [emit_conv1_chunk(1, g * GSZ + j, j)
                        for j in range(GSZ)]
                emit_conv2_group(1, g, c1ts, last=(g == NG - 1))

    nc.compile()
    _BUILT = nc
    return nc


def _prep_host(inputs):
    import ml_dtypes
    bf16 = ml_dtypes.bfloat16
    x = np.asarray(inputs["x"], np.float32)
    Wq = np.asarray(inputs["Wq"], np.float32)
    bq = np.asarray(inputs["bq"], np.float32)
    Wk = np.asarray(inputs["Wk"], np.float32)
    bk = np.asarray(inputs["bk"], np.float32)
    Wv = np.asarray(inputs["Wv"], np.float32)
    bv = np.asarray(inputs["bv"], np.float32)
    Wo = np.asarray(inputs["Wo"], np.float32)
    bo = np.asarray(inputs["bo"], np.float32)
    g1 = np.asarray(inputs["g1"], np.float32)
    b1 = np.asarray(inputs["b1"], np.float32)
    g2 = np.asarray(inputs["g2"], np.float32)
    b2 = np.asarray(inputs["b2"], np.float32)
    Wc1 = np.asarray(inputs["Wc1"], np.float32)
    bc1 = np.asarray(inputs["bc1"], np.float32)
    Wc2 = np.asarray(inputs["Wc2"], np.float32)
    bc2 = np.asarray(inputs["bc2"], np.float32)

    xT = np.ascontiguousarray(x.transpose(0, 2, 1).reshape(B, NDC, 128, S)) \
           .astype(bf16)
    xp = np.ascontiguousarray((x + bo[None, None, :]).reshape(B, NSC, 128, D))

    wqk = np.zeros((2, 4, 128, 512), np.float32)
    for proj, W in ((0, Wq), (1, Wk)):
        for pair in range(4):
            blk = np.concatenate([W[2 * pair], W[2 * pair + 1]], axis=1)
            wqk[proj, pair] = blk.reshape(NDC, 128, 128).transpose(1, 0, 2) \
                                 .reshape(128, 512)
    bqk = np.zeros((128, 8), np.float32)
    for proj, b in ((0, bq), (1, bk)):
        for pair in range(4):
            bqk[:, proj * 4 + pair] = np.concatenate(
                [b[2 * pair], b[2 * pair + 1]])

    wv = np.zeros((NDC, 128, 520), np.float32)
    bvrow = np.zeros((128, 520), np.float32)
    for h in range(H):
        wv[:, :, h * 65:h * 65 + 64] = Wv[h].reshape(NDC, 128, 64)
        bvrow[0, h * 65:h * 65 + 64] = bv[h]
        bvrow[0, h * 65 + 64] = 1.0

    wo = np.ascontiguousarray(Wo.reshape(4, 128, 512))

    w1 = np.ascontiguousarray(
        Wc1.reshape(NCD, 128, NDC, 128, KS).transpose(0, 2, 3, 4, 1)
           .reshape(NCD, NDC, 128, KS * 128)).astype(bf16)
    w2 = np.ascontiguousarray(
        Wc2.reshape(D, NCD, 128, KS).transpose(1, 2, 3, 0)
           .reshape(NCD, 128, KS * 512)).astype(bf16)
    bc1s = np.ascontiguousarray(bc1.reshape(NCD, 128).T)

    gb = np.stack([np.tile(v[None, :], (128, 1))
                   for v in (g1, b1, g2, b2, bc2)]).astype(bf16)
    cones = np.ones((128, 128), bf16)
    czero = np.zeros((128, 8), bf16)

    shared = dict(wqk=wqk.astype(bf16), bqk=bqk, wv=wv.astype(bf16),
                  bvrow=bvrow.astype(bf16), wo=wo.astype(bf16),
                  w1=w1, w2=w2, bc1s=bc1s, gb=gb, cones=cones, czero=czero)
    in_maps = []
    for c in range(NCORES):
        m = dict(shared)
        m["xT"] = np.ascontiguousarray(xT[c * NIT:(c + 1) * NIT])
        m["xp"] = np.ascontiguousarray(xp[c * NIT:(c + 1) * NIT])
        in_maps.append(m)
    return in_maps


def run(inputs, trace=False, **trace_kwargs):
    nc = _build()
    from concourse.bass_utils import run_bass_kernel_spmd
    in_maps = _prep_host(inputs)
    res = run_bass_kernel_spmd(nc, in_maps, core_ids=list(range(NCORES)),
                               trace=trace, **trace_kwargs)
    y = np.concatenate([res.results[c]["y"].reshape(NIT, S, D)
                        for c in range(NCORES)], axis=0)
    return y, res


def kernel(**inputs):
    y, _ = run(inputs, trace=False)
    return y



# revision 29
# speedup vs baseline: 1.0624x; 1.0160x over previous
"""Trainium2 Bass kernel for the FFT-block (attention + conv FFN) problem.

Sharding: data-parallel over batch. B=16 items across 8 cores -> 2 items/core.
Each core runs the full block for its items; no collectives.

Per item:
  - attention via scores^T = K Q^T (softmax sums land on the partition axis and
    are folded into the ctx matmul through a ones-column appended to V); the
    per-head 1/Z normalization is broadcast across partitions with a K=1 PE
    matmul.  Attention matmuls run in fp32r (tf32-like, fp32 accumulate);
    softmax weights and V are bf16.
  - convs are 9 shifted matmuls over transposed activations hT [D, S_pad] in
    bf16 (weights+activations), fp32 PSUM accumulation and fp32 o2 accumulator.
  - emission order software-pipelines item1's attention into item0's conv
    stream so the PE never drains (HAM stays at K=8/8).
"""
import sys, types
import numpy as np

B, S, D = 16, 1024, 512
H, DK = 8, 64
CD, KS = 2048, 9
EPS = 1e-5
NCORES = 8
NIT = B // NCORES
NDC = D // 128             # 4 d-chunks
NSC = S // 128             # 8 s-chunks
NCOL = S // 512            # 2 s-cols
NCD = CD // 128            # 16 cd-chunks


def _install_ntff_hook():
    try:
        from antenv.axon_hooks import get_axon_ntff_profile_hook  # noqa
        return
    except ImportError:
        pass
    try:
        from trn_agent_boot.trn_boot import _ntff_profile_via_ctypes
        mod = types.ModuleType('antenv.axon_hooks')
        hook = _ntff_profile_via_ctypes('/opt/axon/libaxon_pjrt.so')
        mod.get_axon_ntff_profile_hook = lambda: hook
        sys.modules['antenv.axon_hooks'] = mod
    except Exception:
        pass


_BUILT = None


def _build():
    global _BUILT
    if _BUILT is not None:
        return _BUILT
    _install_ntff_hook()
    import concourse.bacc as bacc
    import concourse.mybir as mybir
    from concourse import tile
    from concourse.masks import make_identity
    from contextlib import ExitStack

    from collections import deque
    F32 = mybir.dt.float32
    F32R = mybir.dt.float32r
    BF16 = mybir.dt.bfloat16
    F8 = mybir.dt.float8e4
    AF = mybir.ActivationFunctionType
    ALU = mybir.AluOpType
    AX = mybir.AxisListType

    nc = bacc.Bacc("TRN2", target_bir_lowering=False, debug=False,
                   num_devices=NCORES)

    # ---- DRAM I/O (per core) ----
    d_xT = nc.dram_tensor("xT", [NIT, NDC, 128, S], BF16, kind="ExternalInput")
    d_xp = nc.dram_tensor("xp", [NIT, NSC, 128, D], F32, kind="ExternalInput")
    d_wqk = nc.dram_tensor("wqk", [2, 4, 128, 512], BF16, kind="ExternalInput")
    d_bqk = nc.dram_tensor("bqk", [128, 8], F32, kind="ExternalInput")
    d_wv = nc.dram_tensor("wv", [NDC, 128, 520], BF16, kind="ExternalInput")
    d_bvrow = nc.dram_tensor("bvrow", [128, 520], BF16, kind="ExternalInput")
    d_wo = nc.dram_tensor("wo", [4, 128, 512], BF16, kind="ExternalInput")
    d_w1 = nc.dram_tensor("w1", [NCD, NDC, 128, KS * 128], BF16,
                          kind="ExternalInput")
    d_w2 = nc.dram_tensor("w2", [NCD, 128, KS * 512], BF16,
                          kind="ExternalInput")
    d_bc1s = nc.dram_tensor("bc1s", [128, NCD], F32, kind="ExternalInput")
    d_gb = nc.dram_tensor("gb", [5, 128, 512], BF16, kind="ExternalInput")
    d_cones = nc.dram_tensor("cones", [128, 128], BF16, kind="ExternalInput")
    d_czero = nc.dram_tensor("czero", [128, 8], BF16, kind="ExternalInput")
    d_y = nc.dram_tensor("y", [NIT, NSC, 128, D], F32, kind="ExternalOutput")

    G1, B1, G2, B2, BC2 = range(5)

    with tile.TileContext(nc) as tc:
        est = ExitStack()
        with est:
            cp = est.enter_context(tc.tile_pool(name="const", bufs=1))
            pl = est.enter_context(tc.tile_pool(name="work", bufs=1))
            ps = est.enter_context(tc.tile_pool(name="psum", bufs=1, space="PSUM"))
            dp = est.enter_context(tc.tile_pool(name="dramp", bufs=1, space="DRAM"))

            h_dram = [[dp.tile([128, D], F32, tag=f"hd{it}_{sc}",
                               name=f"hd{it}_{sc}")
                       for sc in range(NSC)] for it in range(NIT)]

            # ---- PE warm-up + prioritized x DMA ----
            # ident needs no DMA; junk matmuls keep the PE busy (HAM K=8/8)
            # while the first x/weight DMAs land.
            t_ident = cp.tile([128, 128], F32, tag="ident")
            make_identity(nc, t_ident[:])
            t_identb = cp.tile([128, 128], BF16, tag="identb")
            nc.vector.tensor_copy(t_identb[:], t_ident[:])
            # x DMA for item 0 queued before all const DMAs
            xt0 = []
            for dc in range(NDC):
                t = pl.tile([128, S], BF16, tag=f"xt{dc}", name=f"xt{dc}")
                nc.sync.dma_start(t[:], d_xT[0, dc])
                xt0.append(t)
            pwu = ps.tile([128, 512], F32, tag="pp", bufs=4)
            for _ in range(100):
                nc.tensor.matmul(pwu[:, 0:128], t_identb[:], t_identb[:],
                                 start=True, stop=True)

            # ---- constants ----
            t_bqk = cp.tile([128, 8], F32, tag="bqk")
            nc.sync.dma_start(t_bqk[:], d_bqk[:])
            t_gb = []
            for i in range(5):
                t = cp.tile([128, 512], BF16, tag=f"gb{i}", name=f"gb{i}")
                nc.sync.dma_start(t[:], d_gb[i])
                t_gb.append(t)
            t_bc1s = cp.tile([128, NCD], F32, tag="bc1s")
            nc.sync.dma_start(t_bc1s[:], d_bc1s[:])
            t_cones = cp.tile([128, 128], BF16, tag="cones")
            nc.sync.dma_start(t_cones[:], d_cones[:])
            t_czero = cp.tile([128, 8], BF16, tag="czero")
            nc.sync.dma_start(t_czero[:], d_czero[:])
            t_eps = cp.tile([128, 1], F32, tag="eps")
            nc.vector.memset(t_eps[:], EPS)
            t_neg3 = cp.tile([128, 1], F32, tag="neg3")
            nc.vector.memset(t_neg3[:], -3.0)
            t_wv = []
            for dc in range(NDC):
                t = cp.tile([128, 520], BF16, tag=f"wv{dc}", name=f"wv{dc}")
                nc.sync.dma_start(t[:], d_wv[dc])
                t_wv.append(t)
            t_bvrow = cp.tile([128, 520], BF16, tag="bvrow")
            nc.sync.dma_start(t_bvrow[:], d_bvrow[:])
            t_wo = []
            for c in range(4):
                t = cp.tile([128, 512], BF16, tag=f"wo{c}", name=f"wo{c}")
                nc.sync.dma_start(t[:], d_wo[c])
                t_wo.append(t)

            # persistent hT tiles (bf16, padded s)
            hT = [[pl.tile([128, S + 8], BF16, tag=f"ht{it}_{dc}",
                           name=f"ht{it}_{dc}")
                   for dc in range(NDC)] for it in range(NIT)]

            state = [dict() for _ in range(NIT)]

            # ================= emit helpers =================
            def emit_x(it):
                st = state[it]
                if it == 0:
                    st["xt"] = xt0
                else:
                    xt = []
                    for dc in range(NDC):
                        t = pl.tile([128, S], BF16, tag=f"xt{dc}",
                                    name=f"xt{dc}")
                        nc.sync.dma_start(t[:], d_xT[it, dc])
                        xt.append(t)
                    st["xt"] = xt
                st["qkt"] = {}

            DR = mybir.MatmulPerfMode.DoubleRow

            def v_units(it):
                """V projection closures, one per (tc, half). V lands in fp8
                vstall [128, tc(8), h(8), 66]: 65 cols per head (64 dk + ones
                marker) padded to 66 so the DR middle step (528B) is 16B
                aligned."""
                st = state[it]
                xt = st["xt"]
                vsa = pl.tile([128, NSC, 8, 66], F8, tag=f"vsa{it}",
                              name=f"vsa{it}")
                st["vst"] = vsa
                units = []

                def mk(tc_i, half):
                    def emit():
                        colo = half * 260
                        pv = ps.tile([128, 260], F32, tag="pp", bufs=4)
                        for dc in range(NDC):
                            nc.tensor.matmul(
                                pv[:], xt[dc][:, tc_i * 128:(tc_i + 1) * 128],
                                t_wv[dc][:, colo:colo + 260],
                                start=(dc == 0), stop=False)
                        nc.tensor.matmul(
                            pv[:], t_cones[0:1, 0:128],
                            t_bvrow[0:1, colo:colo + 260],
                            start=False, stop=True)
                        nc.vector.tensor_copy(
                            vsa[:, tc_i, half * 4:(half + 1) * 4, 0:65],
                            pv[:])
                    return emit
                for tc_i in range(NSC):
                    for half in range(2):
                        units.append(mk(tc_i, half))
                return units

            def qk_units(it, pair):
                """Q/K projection closures, one per (proj, scol)."""
                st = state[it]
                xt = st["xt"]
                units = []

                def mk(proj, scol, wt, qt):
                    def emit():
                        if scol == 0:
                            nc.sync.dma_start(wt[:], d_wqk[proj, pair])
                        pq = ps.tile([128, 512], F32, tag="pp", bufs=4)
                        for dc in range(NDC):
                            nc.tensor.matmul(
                                pq[:], wt[:, dc * 128:(dc + 1) * 128],
                                xt[dc][:, scol * 512:(scol + 1) * 512],
                                start=(dc == 0), stop=(dc == NDC - 1))
                        nc.vector.tensor_scalar_add(
                            qt[:, scol * 512:(scol + 1) * 512], pq[:],
                            t_bqk[:, proj * 4 + pair:proj * 4 + pair + 1])
                    return emit
                for proj in range(2):
                    wt = pl.tile([128, 512], BF16, tag=f"wqk{proj}",
                                 bufs=2, name="wt")
                    qt = pl.tile([128, S], BF16, tag=f"qk{proj}{pair}",
                                 name="qt")
                    st["qkt"][(proj, pair)] = qt
                    for scol in range(NCOL):
                        units.append(mk(proj, scol, wt, qt))
                return units

            def emit_heads_pair(it, pair, fillers=None):
                """Both heads of the pair. Scores for the two heads run
                concurrently via PE row-tiling; exp writes fp8 tiles; ctx is
                a DoubleRow fp8 matmul over ti-pairs, woven into the scores
                stream so the PE never waits a full exp latency.  `fillers`
                is a deque of independent emission closures popped between
                score steps to cover the exp drain."""
                st = state[it]
                if pair == 0:
                    st["ctxT"] = [pl.tile([128, S], BF16, tag=f"ct{c}",
                                          name=f"ct{c}") for c in range(4)]
                qT = st["qkt"][(0, pair)]
                kT = st["qkt"][(1, pair)]
                vsa = st["vst"]
                ctxT = st["ctxT"]

                def drain(pc, sub):
                    hr = slice(sub * 64, sub * 64 + 64)
                    zr = pl.tile([64, 512], BF16, tag="zrt", bufs=2,
                                 name="zr")
                    nc.vector.tensor_copy(zr[0:1, :], pc[64:65, :])
                    craw = pl.tile([64, 512], BF16, tag="craw", bufs=2,
                                   name="craw")
                    nc.vector.tensor_copy(craw[:], pc[0:64, :])
                    pb = ps.tile([64, 512], F32, tag="pp", bufs=4)
                    nc.tensor.matmul(pb[:], t_cones[0:1, 0:64], zr[0:1, :],
                                     start=True, stop=True)
                    bcs = pl.tile([64, 512], F32, tag="bcs", bufs=2,
                                  name="bcs")
                    nc.vector.reciprocal_approx_fast(out=bcs[:], in_=pb[:])
                    nc.vector.tensor_tensor(
                        ctxT[pair][hr, so:so + 512], craw[:],
                        bcs[:], ALU.mult)

                for scol in range(NCOL):
                    so = scol * 512
                    pex = [pl.tile([128, NSC, 512], F8, tag=f"pexA{sub}",
                                   bufs=1, name="pe") for sub in range(2)]
                    pc0 = ps.tile([65, 512], F32, tag="pc", bufs=1)
                    h0 = 2 * pair
                    for ti in range(NSC):
                        for sub in range(2):
                            hr = slice(sub * 64, sub * 64 + 64)
                            pp = ps.tile([128, 512], F32, tag="pp", bufs=4)
                            nc.tensor.matmul(
                                pp[:], kT[hr, ti * 128:(ti + 1) * 128],
                                qT[hr, so:so + 512], start=True, stop=True)
                            nc.scalar.activation(pex[sub][:, ti, :], pp[:],
                                                 AF.Exp, bias=t_neg3[:],
                                                 scale=0.125)
                        if ti % 2 == 1:
                            # ctx for head sub0 over the (ti-1, ti) pair
                            nc.tensor.matmul(
                                pc0[:],
                                vsa[:, ti - 1:ti + 1, h0, 0:65],
                                pex[0][:, ti - 1:ti + 1, :],
                                start=(ti == 1), stop=(ti == NSC - 1),
                                perf_mode=DR)
                            if fillers:
                                fillers.popleft()()
                    drain(pc0, 0)
                    pc1 = ps.tile([65, 512], F32, tag="pc", bufs=1)
                    for tj in range(NSC // 2):
                        nc.tensor.matmul(
                            pc1[:],
                            vsa[:, 2 * tj:2 * tj + 2, h0 + 1, 0:65],
                            pex[1][:, 2 * tj:2 * tj + 2, :],
                            start=(tj == 0), stop=(tj == NSC // 2 - 1),
                            perf_mode=DR)
                    drain(pc1, 1)


            def emit_tail(it):
                """Wo + residual + LN1 + transpose into hT (+ h spill).
                Fully per-sc (LN rows are independent s positions)."""
                st = state[it]
                ctxT = st["ctxT"]
                for sc in range(NSC):
                    xpt = pl.tile([128, 512], F32, tag="xpt", bufs=2)
                    nc.sync.dma_start(xpt[:], d_xp[it, sc])
                    pw = ps.tile([128, 512], F32, tag="pc", bufs=1)
                    for c in range(4):
                        nc.tensor.matmul(
                            pw[:], ctxT[c][:, sc * 128:(sc + 1) * 128],
                            t_wo[c][:], start=(c == 0), stop=(c == 3))
                    r = pl.tile([128, 512], F32, tag="res_t", bufs=2, name="r")
                    nc.vector.tensor_tensor(r[:], pw[:], xpt[:], ALU.add)
                    st1 = pl.tile([128, 2], F32, tag="st1", bufs=3)
                    nc.vector.reduce_sum(st1[:, 0:1], r[:], axis=AX.X)
                    sq = pl.tile([128, 512], BF16, tag="sqs", bufs=2, name="sq")
                    nc.scalar.activation(sq[:], r[:], AF.Square,
                                         accum_out=st1[:, 1:2])
                    mv = pl.tile([128, 2], F32, tag="mv1", bufs=3)
                    nc.vector.tensor_scalar_mul(mv[:], st1[:], 1.0 / D)
                    inv1 = pl.tile([128, 1], F32, tag="inv1", bufs=3)
                    nc.vector.tensor_tensor(inv1[:], mv[:, 0:1], mv[:, 0:1],
                                            ALU.mult)
                    nc.vector.tensor_tensor(inv1[:], mv[:, 1:2], inv1[:],
                                            ALU.subtract)
                    nc.scalar.activation(inv1[:], inv1[:], AF.Sqrt,
                                         bias=t_eps[:])
                    nc.vector.reciprocal(inv1[:], inv1[:])
                    ht_ = pl.tile([128, 512], F32, tag="hst", bufs=2, name="h_")
                    nc.vector.tensor_scalar(
                        ht_[:], r[:], mv[:, 0:1], inv1[:, 0:1],
                        ALU.subtract, ALU.mult)
                    nc.vector.tensor_tensor(ht_[:], ht_[:], t_gb[G1][:],
                                            ALU.mult)
                    nc.vector.tensor_tensor(ht_[:], ht_[:], t_gb[B1][:],
                                            ALU.add)
                    nc.sync.dma_start(h_dram[it][sc][:], ht_[:])
                    for dc in range(NDC):
                        pt = ps.tile([128, 128], F32, tag="pp", bufs=4)
                        nc.tensor.transpose(pt[:], ht_[:, dc * 128:(dc + 1) * 128],
                                            t_ident[:])
                        nc.vector.tensor_copy(
                            hT[it][dc][:, 4 + sc * 128: 4 + (sc + 1) * 128],
                            pt[:])
                for dc in range(NDC):
                    nc.sync.dma_start(hT[it][dc][:, 0:4], d_czero[:, 0:4])
                    nc.sync.dma_start(hT[it][dc][:, S + 4:S + 8],
                                      d_czero[:, 4:8])

            o2 = [[None] * NSC for _ in range(NIT)]
            GSZ = 4                       # cdc chunks per conv2 group
            NG = NCD // GSZ               # 4 groups

            def emit_conv1_chunk(it, cdc, slot):
                """conv1 for one cdc chunk -> c1t tile (slot 0..3 in group)."""
                w1t = []
                for dc in range(NDC):
                    t = pl.tile([128, KS * 128], BF16, tag=f"w1t{dc}", bufs=2,
                                name="w1t")
                    nc.sync.dma_start(t[:], d_w1[cdc, dc])
                    w1t.append(t)
                c1t = pl.tile([128, S + 8], BF16, tag=f"c1t{slot}", bufs=2,
                              name="c1t")
                nc.sync.dma_start(c1t[:, 0:4], d_czero[:, 0:4])
                nc.sync.dma_start(c1t[:, S + 4:S + 8], d_czero[:, 4:8])
                for scol in range(NCOL):
                    pc1 = ps.tile([128, 512], F32, tag="cv", bufs=3)
                    idx = 0
                    for k in range(KS):
                        for dc in range(NDC):
                            nc.tensor.matmul(
                                pc1[:], w1t[dc][:, k * 128:(k + 1) * 128],
                                hT[it][dc][:, scol * 512 + k:
                                           scol * 512 + k + 512],
                                start=(idx == 0), stop=(idx == 35))
                            idx += 1
                    nc.scalar.activation(
                        c1t[:, 4 + scol * 512: 4 + (scol + 1) * 512],
                        pc1[:], AF.Relu, bias=t_bc1s[:, cdc:cdc + 1])
                return c1t

            def emit_ln2_sc(it, sc):
                """Per-s-chunk LN2: fully independent per row -> no batching."""
                t1 = pl.tile([128, 512], F32, tag="hst", bufs=2)
                nc.vector.tensor_tensor(t1[:], o2[it][sc][:], t_gb[BC2][:],
                                        ALU.add)
                nc.scalar.activation(t1[:], t1[:], AF.Relu)
                hrl = pl.tile([128, 512], F32, tag="xpt", bufs=2)
                nc.sync.dma_start(hrl[:], h_dram[it][sc][:])
                r = pl.tile([128, 512], F32, tag="res_ln2", bufs=2, name="r2")
                nc.vector.tensor_tensor(r[:], t1[:], hrl[:], ALU.add)
                st1 = pl.tile([128, 2], F32, tag="st1", bufs=3)
                nc.vector.reduce_sum(st1[:, 0:1], r[:], axis=AX.X)
                sq = pl.tile([128, 512], BF16, tag="sqs", bufs=2, name="sq2")
                nc.scalar.activation(sq[:], r[:], AF.Square,
                                     accum_out=st1[:, 1:2])
                mv = pl.tile([128, 2], F32, tag="mv1", bufs=3)
                nc.vector.tensor_scalar_mul(mv[:], st1[:], 1.0 / D)
                inv1 = pl.tile([128, 1], F32, tag="inv1", bufs=3)
                nc.vector.tensor_tensor(inv1[:], mv[:, 0:1], mv[:, 0:1],
                                        ALU.mult)
                nc.vector.tensor_tensor(inv1[:], mv[:, 1:2], inv1[:],
                                        ALU.subtract)
                nc.scalar.activation(inv1[:], inv1[:], AF.Sqrt, bias=t_eps[:])
                nc.vector.reciprocal(inv1[:], inv1[:])
                yt = pl.tile([128, 512], F32, tag="hst", bufs=2)
                nc.vector.tensor_scalar(
                    yt[:], r[:], mv[:, 0:1], inv1[:, 0:1],
                    ALU.subtract, ALU.mult)
                nc.vector.tensor_tensor(yt[:], yt[:], t_gb[G2][:], ALU.mult)
                nc.vector.tensor_tensor(yt[:], yt[:], t_gb[B2][:], ALU.add)
                nc.sync.dma_start(d_y[it, sc], yt[:])

            def emit_conv2_group(it, g, c1ts, last):
                """conv2 accumulated over a 4-cdc group in PSUM; on the last
                group, fuse per-sc LN2 right after each sc completes."""
                w2t = []
                for j in range(GSZ):
                    t = pl.tile([128, KS * 512], BF16, tag=f"w2t{j}", bufs=1,
                                name="w2t")
                    nc.sync.dma_start(t[:], d_w2[g * GSZ + j])
                    w2t.append(t)
                for sc in range(NSC):
                    pc2 = ps.tile([128, 512], F32, tag="cv", bufs=3)
                    idx = 0
                    for j in range(GSZ):
                        for k in range(KS):
                            nc.tensor.matmul(
                                pc2[:],
                                c1ts[j][:, sc * 128 + k: sc * 128 + k + 128],
                                w2t[j][:, k * 512:(k + 1) * 512],
                                start=(idx == 0), stop=(idx == GSZ * KS - 1))
                            idx += 1
                    if g == 0:
                        t = pl.tile([128, 512], F32, tag=f"o2_{sc}",
                                    bufs=1, name=f"o2_{sc}")
                        o2[it][sc] = t
                        nc.vector.tensor_copy(t[:], pc2[:])
                    else:
                        nc.vector.tensor_tensor(o2[it][sc][:], pc2[:],
                                                o2[it][sc][:], ALU.add)
                    if last:
                        emit_ln2_sc(it, sc)

            # ================= emission order =================
            emit_x(0)
            for u in v_units(0):
                u()
            for pair in range(4):
                for u in qk_units(0, pair):
                    u()
            emit_x(1)
            fillers = deque(v_units(1))
            for pair in range(4):
                fillers.extend(qk_units(1, pair))
            for pair in range(4):
                emit_heads_pair(0, pair, fillers)
            while fillers:
                fillers.popleft()()
            emit_tail(0)
            for g in range(NG):
                c1ts = [emit_conv1_chunk(0, g * GSZ + j, j)
                        for j in range(GSZ)]
                if g < 2:
                    emit_heads_pair(1, 2 * g)
                    emit_heads_pair(1, 2 * g + 1)
                emit_conv2_group(0, g, c1ts, last=(g == NG - 1))
                if g == 1:
                    emit_tail(1)
            for g in range(NG):
                c1ts = [emit_conv1_chunk(1, g * GSZ + j, j)
                        for j in range(GSZ)]
                emit_conv2_group(1, g, c1ts, last=(g == NG - 1))

    nc.compile()
    _BUILT = nc
    return nc


def _prep_host(inputs):
    import ml_dtypes
    bf16 = ml_dtypes.bfloat16
    x = np.asarray(inputs["x"], np.float32)
    Wq = np.asarray(inputs["Wq"], np.float32)
    bq = np.asarray(inputs["bq"], np.float32)
    Wk = np.asarray(inputs["Wk"], np.float32)
    bk = np.asarray(inputs["bk"], np.float32)
    Wv = np.asarray(inputs["Wv"], np.float32)
    bv = np.asarray(inputs["bv"], np.float32)
    Wo = np.asarray(inputs["Wo"], np.float32)
    bo = np.asarray(inputs["bo"], np.float32)
    g1 = np.asarray(inputs["g1"], np.float32)
    b1 = np.asarray(inputs["b1"], np.float32)
    g2 = np.asarray(inputs["g2"], np.float32)
    b2 = np.asarray(inputs["b2"], np.float32)
    Wc1 = np.asarray(inputs["Wc1"], np.float32)
    bc1 = np.asarray(inputs["bc1"], np.float32)
    Wc2 = np.asarray(inputs["Wc2"], np.float32)
    bc2 = np.asarray(inputs["bc2"], np.float32)

    xT = np.ascontiguousarray(x.transpose(0, 2, 1).reshape(B, NDC, 128, S)) \
           .astype(bf16)
    xp = np.ascontiguousarray((x + bo[None, None, :]).reshape(B, NSC, 128, D))

    wqk = np.zeros((2, 4, 128, 512), np.float32)
    for proj, W in ((0, Wq), (1, Wk)):
        for pair in range(4):
            blk = np.concatenate([W[2 * pair], W[2 * pair + 1]], axis=1)
            wqk[proj, pair] = blk.reshape(NDC, 128, 128).transpose(1, 0, 2) \
                                 .reshape(128, 512)
    bqk = np.zeros((128, 8), np.float32)
    for proj, b in ((0, bq), (1, bk)):
        for pair in range(4):
            bqk[:, proj * 4 + pair] = np.concatenate(
                [b[2 * pair], b[2 * pair + 1]])

    wv = np.zeros((NDC, 128, 520), np.float32)
    bvrow = np.zeros((128, 520), np.float32)
    for h in range(H):
        wv[:, :, h * 65:h * 65 + 64] = Wv[h].reshape(NDC, 128, 64)
        bvrow[0, h * 65:h * 65 + 64] = bv[h]
        bvrow[0, h * 65 + 64] = 1.0

    wo = np.ascontiguousarray(Wo.reshape(4, 128, 512))

    w1 = np.ascontiguousarray(
        Wc1.reshape(NCD, 128, NDC, 128, KS).transpose(0, 2, 3, 4, 1)
           .reshape(NCD, NDC, 128, KS * 128)).astype(bf16)
    w2 = np.ascontiguousarray(
        Wc2.reshape(D, NCD, 128, KS).transpose(1, 2, 3, 0)
           .reshape(NCD, 128, KS * 512)).astype(bf16)
    bc1s = np.ascontiguousarray(bc1.reshape(NCD, 128).T)

    gb = np.stack([np.tile(v[None, :], (128, 1))
                   for v in (g1, b1, g2, b2, bc2)]).astype(bf16)
    cones = np.ones((128, 128), bf16)
    czero = np.zeros((128, 8), bf16)

    shared = dict(wqk=wqk.astype(bf16), bqk=bqk, wv=wv.astype(bf16),
                  bvrow=bvrow.astype(bf16), wo=wo.astype(bf16),
                  w1=w1, w2=w2, bc1s=bc1s, gb=gb, cones=cones, czero=czero)
    in_maps = []
    for c in range(NCORES):
        m = dict(shared)
        m["xT"] = np.ascontiguousarray(xT[c * NIT:(c + 1) * NIT])
        m["xp"] = np.ascontiguousarray(xp[c * NIT:(c + 1) * NIT])
        in_maps.append(m)
    return in_maps


def run(inputs, trace=False, **trace_kwargs):
    nc = _build()
    from concourse.bass_utils import run_bass_kernel_spmd
    in_maps = _prep_host(inputs)
    res = run_bass_kernel_spmd(nc, in_maps, core_ids=list(range(NCORES)),
                               trace=trace, **trace_kwargs)
    y = np.concatenate([res.results[c]["y"].reshape(NIT, S, D)
                        for c in range(NCORES)], axis=0)
    return y, res


def kernel(**inputs):
    y, _ = run(inputs, trace=False)
    return y

